# revision 8
# baseline (speedup 1.0000x reference)
"""Trainium2 Bass kernel for nn_Attention_73289321939579.

Gated attention block (AlphaFold-style):
  qkv = q_x @ w_qkv.T ; q /= sqrt(64)
  scores = q k^T + bias ; attn = softmax(scores, keys)
  o = (attn @ v) * sigmoid(q_x @ w_g.T + b_g)
  out = o @ w_o.T + b_o

Sharding over 8 cores: core = b*4 + qh*2 + hq
  b  = batch (2)            -> data parallel
  qh = query half (2x1024)  -> bias/q sliced, output row-sliced
  hq = head quad (2x4 heads)-> tensor parallel; partial outputs summed on host

Device layout (per core, contractions on the SBUF partition axis; ALL bulk
inputs pre-arranged on host to [128 partitions, contiguous-lines] so each
is ONE DMA with 128 fat descriptors instead of thousands of 1KB ones):
  xt  [128, 4*2048]   bf16 = q_x[b].T key-rolled, channel-chunked
  eb  [128, 2*16*512] bf16 = exp(bias[b,0]).T chunked [key128, ic, jc, q];
        softmax computed as exp(qk) * exp(bias), exact in fp32/bf16
  wt  [128, 4*1024]   bf16 = [wq.T/8 | wk.T | wv.T | wg.T] channel-chunked
  wot [128, 2*512]    bf16 = w_o[:, heads].T pair-major
  bg  [256, 1] f32 = 0.5*b_g[heads]  (gate via 0.5*tanh(0.5x+0.5bg)+0.5;
        Tanh shares the ACT "exp" table set -> no table swap)
  bsel [128,128] f32 = 0/1: row 64 -> out partitions 0:64, row 65 -> 64:128
        (engine writes at partition base 65 are illegal -> host constant)

Schedule. ACT exp of the scores ([128,1024] per step at ~1.0us, no dtype
speedup on ACT) is the hard floor: 64 steps ~= 67us. Everything else hides
under it or under the PE:
  - bf16 operands everywhere: halves DMA/SBUF/LDWEIGHTS traffic, DVE
    multiply runs in 2x mode (~0.69us), PE cycles unchanged
  - one flat software-pipelined stream of 64 (ic,hp,jc) steps: per step
    S-pair matmuls -> exp -> eb-multiply, with the O-accumulate matmuls
    emitted 2 steps behind so the in-order PE never waits on the exp
    pipeline; no pass-boundary stalls
  - PSUM = 2x 2-bank S slots + 2x 1-bank O accumulators + 2x 1-bank
    broadcast/out-proj slots = exactly 8 banks
  - projections: a minimal upfront set (K pair0, Q ic0 pair0, V j0/j1,
    gate) runs before the first step; the remaining ~21 groups ride a
    closure queue, interleaved 2-per-step into the early steps' PE slack
  - each pass's normalize + gate + out-projection is also carried as
    closures into the following steps; denominator row moves ride
    SBUF->SBUF DMAs (partition-base-65 writes are DMA-only anyway)
  - normalize is pair-wise: both heads' denominators assembled in one
    tile, one reciprocal, one fp32 bsel-matmul broadcast
All matmuls are plain 128x128 mode (mode switches drain the PE array).
"""

import sys

for _p in ("/opt/trn_rl_repo",):
    if _p not in sys.path:
        sys.path.insert(0, _p)

from collections import deque
from contextlib import ExitStack

import ml_dtypes
import numpy as np

import concourse.bass as bass  # noqa: F401
import concourse.mybir as mybir
import concourse.tile as tile
from concourse import bacc
from concourse.bass_utils import run_bass_kernel_spmd

# ---- problem dims (hardcoded per contest contract) ----
B, Q, CQ = 2, 2048, 512
H, D = 8, 64
P = 128
QL = 1024          # queries per core
EL = 256           # e-dims per core (4 heads x 64)
HL = 4             # heads per core
CC = CQ // P       # 4 contraction chunks over channels
EC = EL // P       # 2 head-pairs
NJ = Q // P        # 16 key chunks
NI = QL // 512     # 2 query chunks of 512

F32 = mybir.dt.float32
BF16 = mybir.dt.bfloat16
MUL = mybir.AluOpType.mult
ADD = mybir.AluOpType.add
EXP = mybir.ActivationFunctionType.Exp
TANH = mybir.ActivationFunctionType.Tanh

OFF_Q, OFF_K, OFF_V, OFF_G = 0, EL, 2 * EL, 3 * EL


def _emit(tc, xt, eb, wt, wot, bg, bsel, outp):
    nc = tc.nc

    with ExitStack() as ctx:
        const = ctx.enter_context(tc.tile_pool(name="const", bufs=1))
        esp = ctx.enter_context(tc.tile_pool(name="esp", bufs=3))
        ptp = ctx.enter_context(tc.tile_pool(name="ptp", bufs=4))
        workp = ctx.enter_context(tc.tile_pool(name="workp", bufs=2))
        psum = ctx.enter_context(tc.tile_pool(name="psum", bufs=2, space="PSUM"))

        # ---- constants ----
        ones_sb = const.tile([P, P], F32, name="ones_sb", tag="ones_sb")
        nc.vector.memset(ones_sb, 1.0)
        bsel_sb = const.tile([P, P], F32, name="bsel_sb", tag="bsel_sb")
        nc.sync.dma_start(bsel_sb, bsel)
        bg_sb = const.tile([P, EC], F32, name="bg_sb", tag="bg_sb")
        nc.sync.dma_start(bg_sb, bg.rearrange("(o p) u -> p (o u)", p=P))
        woT_sb = const.tile([P, EC, CQ], BF16, name="woT_sb", tag="woT_sb")
        nc.sync.dma_start(woT_sb, wot.rearrange("p (o c) -> p o c", o=EC))
        rec_sbs = []
        for ri in range(2):
            rcb = const.tile([P, 512], F32, name=f"rec_sb{ri}", tag=f"rec_sb{ri}")
            nc.vector.memset(rcb, 0.0)
            rec_sbs.append(rcb)

        # ---- bulk inputs: one fat DMA each (128 descriptors) ----
        wT_sb = const.tile([P, CC, 4 * EL], BF16, name="wT_sb", tag="wT_sb")
        nc.sync.dma_start(wT_sb, wt.rearrange("p (c n) -> p c n", c=CC))
        xT_sb = const.tile([P, CC, Q], BF16, name="xT_sb", tag="xT_sb")
        nc.sync.dma_start(xT_sb, xt.rearrange("p (c q) -> p c q", c=CC))
        # exp(bias), fully resident (scalar HWDGE queue, parallel to the
        # SP-queue input stream)
        ebt_sb = const.tile([P, NI, NJ, 512], BF16, name="ebt_sb", tag="ebt_sb")
        ebr = eb.rearrange("p (i j q) -> p i j q", i=NI, j=NJ)
        for ic in range(NI):
            nc.scalar.dma_start(ebt_sb[:, ic, :, :], ebr[:, ic, :, :])

        # ---- resident intermediates ----
        kT_sb = const.tile([P, EC, Q], BF16, name="kT_sb", tag="kT_sb")
        qTp_sb = const.tile([P, HL, QL], BF16, name="qTp_sb", tag="qTp_sb")
        nc.vector.memset(qTp_sb[64:128, 0::2, :], 0.0)
        nc.vector.memset(qTp_sb[0:64, 1::2, :], 0.0)
        gp_sb = const.tile([P, EC, QL], BF16, name="gp_sb", tag="gp_sb")
        og_sb = const.tile([P, EC, QL], BF16, name="og_sb", tag="og_sb")
        v_sb = const.tile([P, NJ, HL, D + 1], BF16, name="v_sb", tag="v_sb")
        nc.vector.tensor_copy(
            out=v_sb[:, :, :, D],
            in_=ones_sb[:, 0:64].rearrange("p (a b) -> p a b", a=NJ),
        )

        # ---- projection groups (each: 4 accum matmuls + drain) ----
        def proj_k(ec, j4):
            ps_k = psum.tile([P, 512], F32, tag="s", name="ps_k")
            for c in range(CC):
                nc.tensor.matmul(
                    ps_k,
                    wT_sb[:, c, OFF_K + ec * P : OFF_K + (ec + 1) * P],
                    xT_sb[:, c, j4 * 512 : (j4 + 1) * 512],
                    start=(c == 0),
                    stop=(c == CC - 1),
                )
            nc.vector.tensor_copy(
                out=kT_sb[:, ec, j4 * 512 : (j4 + 1) * 512], in_=ps_k
            )

        def proj_q(ic, ec):
            ps_q = psum.tile([P, 512], F32, tag="s", name="ps_q")
            for c in range(CC):
                nc.tensor.matmul(
                    ps_q,
                    wT_sb[:, c, OFF_Q + ec * P : OFF_Q + (ec + 1) * P],
                    xT_sb[:, c, ic * 512 : (ic + 1) * 512],
                    start=(c == 0),
                    stop=(c == CC - 1),
                )
            sl = slice(ic * 512, (ic + 1) * 512)
            nc.vector.tensor_copy(out=qTp_sb[0:64, 2 * ec, sl], in_=ps_q[0:64, :])
            nc.vector.tensor_copy(
                out=qTp_sb[64:128, 2 * ec + 1, sl], in_=ps_q[64:128, :]
            )

        def proj_v(jc):
            ps_v = psum.tile([P, 512], F32, tag="s", name="ps_v")
            for c in range(CC):
                nc.tensor.matmul(
                    ps_v[:, :EL],
                    xT_sb[:, c, jc * P : (jc + 1) * P],
                    wT_sb[:, c, OFF_V : OFF_V + EL],
                    start=(c == 0),
                    stop=(c == CC - 1),
                )
            nc.vector.tensor_copy(
                out=v_sb[:, jc, :, 0:D],
                in_=ps_v[:, :EL].rearrange("p (h d) -> p h d", h=HL),
            )

        def proj_g(ec, ic):
            ps_g = psum.tile([P, 512], F32, tag="s", name="ps_g")
            for c in range(CC):
                nc.tensor.matmul(
                    ps_g,
                    wT_sb[:, c, OFF_G + ec * P : OFF_G + (ec + 1) * P],
                    xT_sb[:, c, ic * 512 : (ic + 1) * 512],
                    start=(c == 0),
                    stop=(c == CC - 1),
                )
            nc.scalar.activation(
                gp_sb[:, ec, ic * 512 : (ic + 1) * 512],
                ps_g,
                TANH,
                bias=bg_sb[:, ec : ec + 1],
                scale=0.5,
            )

        # ---- phase 0: warmup burst (PE clock ramp) over the DMA head ----
        warm_ps = psum.tile([P, 2, 512], F32, tag="s", name="warm_ps")
        for wi in range(14):
            nc.tensor.matmul(
                warm_ps[:, 0, 0:P],
                ones_sb,
                ones_sb,
                start=(wi == 0),
                stop=(wi == 13),
            )
        warm_sb = workp.tile([P, P], F32, name="warm_sb", tag="warm")
        nc.vector.tensor_copy(out=warm_sb[:, 0:P], in_=warm_ps[:, 0, 0:P])

        # ---- phase 1: minimal upfront projections ----
        for j4 in range(Q // 512):
            proj_k(0, j4)
        proj_q(0, 0)
        proj_v(0)
        proj_v(1)
        for ec in range(EC):
            for ic in range(NI):
                proj_g(ec, ic)
        nc.vector.tensor_scalar(gp_sb, gp_sb, 0.5, 0.5, MUL, ADD)

        # ---- phase 2: flat pipelined attention over 64 steps ----
        outr = outp.rearrange("(o p) c -> p o c", p=P)

        def make_tail(ic, hp, o_ps, parity):
            """Normalize + gate closures for one finished (ic,hp) pass."""
            isl = slice(ic * 512, (ic + 1) * 512)
            rec_sb = rec_sbs[parity]
            o_pair = workp.tile([P, 512], F32, name="o_pair", tag="opair")
            wk = workp.tile([P, 512], F32, name="wk", tag="wk")
            recf = workp.tile([P, 512], F32, name="recf", tag="recf")
            ocp = workp.tile([P, 512], BF16, name="ocp", tag="ocp")
            bc_ps = psum.tile([P, 512], F32, tag="op", name="bc_ps")
            ops = []
            # even head -> o_pair rows 0:64 (denom parked in row 64)
            ops.append(
                lambda: nc.vector.tensor_copy(out=o_pair[0:65, :], in_=o_ps[0])
            )
            # odd head -> wk rows 0:64, its denom in wk[64]
            ops.append(lambda: nc.vector.tensor_copy(out=wk[0:65, :], in_=o_ps[1]))

            def _moves():
                # same SP queue: the row read of o_pair[64] completes before
                # the block write overwrites it
                nc.sync.dma_start(wk[65:66, :], o_pair[64:65, :])
                nc.sync.dma_start(o_pair[64:128, :], wk[0:64, :])

            ops.append(_moves)
            ops.append(
                lambda: nc.vector.reciprocal_approx_fast(
                    out=recf[0:66, :], in_=wk[0:66, :]
                )
            )

            def _recrows():
                # recf[64] = 1/denom_odd -> rec row 65; recf[65] -> row 64
                nc.sync.dma_start(rec_sb[65:66, :], recf[64:65, :])
                nc.sync.dma_start(rec_sb[64:65, :], recf[65:66, :])

            ops.append(_recrows)
            ops.append(
                lambda: nc.tensor.matmul(
                    bc_ps, bsel_sb, rec_sb, start=True, stop=True
                )
            )
            ops.append(lambda: nc.vector.tensor_tensor(ocp, bc_ps, o_pair, MUL))
            ops.append(
                lambda: nc.vector.tensor_tensor(
                    og_sb[:, hp, isl], ocp, gp_sb[:, hp, isl], MUL
                )
            )
            return ops

        def make_outproj(ic):
            """Out-projection + store closures for one query block."""
            ops = []
            for ip4 in range(4):
                ip = ic * 4 + ip4
                ps_o = psum.tile([P, 512], F32, tag="op", name="ps_o")
                out_sb = workp.tile([P, 512], F32, name="out_sb", tag="outsb")

                def _mm(ps_o=ps_o, ip=ip):
                    for ec in range(EC):
                        nc.tensor.matmul(
                            ps_o,
                            og_sb[:, ec, ip * P : (ip + 1) * P],
                            woT_sb[:, ec, :],
                            start=(ec == 0),
                            stop=(ec == EC - 1),
                        )

                def _st(ps_o=ps_o, out_sb=out_sb, ip=ip):
                    nc.vector.tensor_copy(out=out_sb, in_=ps_o)
                    nc.sync.dma_start(outr[:, ip, :], out_sb)

                ops.append(_mm)
                ops.append(_st)
            return ops

        # remaining projections ride the closure queue (V first: v[jc] is
        # needed by the O-matmul of flat-step jc+2, consumed 2-per-step)
        tailq = deque()
        for jc in range(2, NJ):
            tailq.append(lambda jc=jc: proj_v(jc))
        for j4 in range(Q // 512):
            tailq.append(lambda j4=j4: proj_k(1, j4))
        tailq.append(lambda: proj_q(0, 1))
        tailq.append(lambda: proj_q(1, 0))
        tailq.append(lambda: proj_q(1, 1))

        PASSES = [(0, 0), (0, 1), (1, 0), (1, 1)]
        steps = [(ic, hp, jc) for (ic, hp) in PASSES for jc in range(NJ)]
        DEPTH = 2
        o_ps_by_pass = {}
        pts = {}

        def emit_O(fs):
            ic, hp, jc = steps[fs]
            o_ps = o_ps_by_pass[(ic, hp)]
            pt = pts.pop(fs)
            for hh in range(2):
                nc.tensor.matmul(
                    o_ps[hh],
                    v_sb[:, jc, 2 * hp + hh, :],
                    pt[:, hh, :],
                    start=(jc == 0),
                    stop=(jc == NJ - 1),
                )
            if jc == NJ - 1:
                tailq.extend(make_tail(ic, hp, o_ps, parity=(2 * ic + hp) % 2))
                if hp == 1:
                    tailq.extend(make_outproj(ic))

        for fs, (ic, hp, jc) in enumerate(steps):
            if jc == 0:
                o_ps_by_pass[(ic, hp)] = [
                    psum.tile(
                        [D + 1, 512], F32, tag="o", name=f"o_ps{hh}", bufs=2
                    )
                    for hh in range(2)
                ]
            isl = slice(ic * 512, (ic + 1) * 512)
            s_ps = psum.tile([P, 2, 512], F32, tag="s", name="s_ps")
            for hh in range(2):
                nc.tensor.matmul(
                    s_ps[:, hh, :],
                    kT_sb[:, hp, jc * P : (jc + 1) * P],
                    qTp_sb[:, 2 * hp + hh, isl],
                    start=True,
                    stop=True,
                )
            es = esp.tile([P, 2, 512], BF16, name="es", tag="es")
            nc.scalar.activation(es, s_ps, EXP)
            pt = ptp.tile([P, 2, 512], BF16, name="pt", tag="pt")
            ebb = ebt_sb[:, ic, jc, :][:, None, :].to_broadcast([P, 2, 512])
            nc.vector.tensor_tensor(pt, es, ebb, MUL)
            pts[fs] = pt
            for _ in range(2):
                if tailq:
                    tailq.popleft()()
            if fs >= DEPTH:
                emit_O(fs - DEPTH)
        for fs in range(len(steps) - DEPTH, len(steps)):
            emit_O(fs)
        while tailq:
            tailq.popleft()()


_CACHE = {}


def _get_nc():
    if "nc" not in _CACHE:
        nc = bacc.Bacc("TRN2", debug=False, enable_asserts=False)
        xt = nc.dram_tensor("xt_in", [P, CC * Q], BF16, kind="ExternalInput").ap()
        eb = nc.dram_tensor(
            "eb_in", [P, NI * NJ * 512], BF16, kind="ExternalInput"
        ).ap()
        wt = nc.dram_tensor("wt_in", [P, CC * 1024], BF16, kind="ExternalInput").ap()
        wot = nc.dram_tensor("wot_in", [P, EC * CQ], BF16, kind="ExternalInput").ap()
        bg = nc.dram_tensor("bg_in", [EL, 1], F32, kind="ExternalInput").ap()
        bsel = nc.dram_tensor("bsel_in", [P, P], F32, kind="ExternalInput").ap()
        outp = nc.dram_tensor("out", [QL, CQ], F32, kind="ExternalOutput").ap()
        with tile.TileContext(nc) as tc:
            _emit(tc, xt, eb, wt, wot, bg, bsel, outp)
        nc.compile()
        _CACHE["nc"] = nc
    return _CACHE["nc"]


LAST_RESULTS = None
BF = ml_dtypes.bfloat16
_BSEL = np.zeros((P, P), np.float32)
_BSEL[64, 0:64] = 1.0
_BSEL[65, 64:128] = 1.0


def _chunk128(a, nchunk):
    """[nchunk*128, L] -> [128, nchunk*L] partition-contiguous layout."""
    n, L = a.shape
    assert n == nchunk * P
    return a.reshape(nchunk, P, L).transpose(1, 0, 2).reshape(P, nchunk * L)


def kernel(q_x, kv_x, bias, w_qkv, w_o, b_o, w_g, b_g):
    global LAST_RESULTS
    q_x = np.asarray(q_x, np.float32)
    bias = np.asarray(bias, np.float32)
    w_qkv = np.asarray(w_qkv, np.float32)
    w_o = np.asarray(w_o, np.float32)
    b_o = np.asarray(b_o, np.float32)
    w_g = np.asarray(w_g, np.float32)
    b_g = np.asarray(b_g, np.float32)

    in_maps = []
    for core in range(8):
        b, qh, hq = core >> 2, (core >> 1) & 1, core & 1
        i0 = qh * QL
        esl = slice(hq * EL, (hq + 1) * EL)
        xTb = q_x[b].T  # [512, 2048]
        # roll keys so this core's queries are columns 0:QL
        xTp = np.concatenate([xTb[:, i0:], xTb[:, :i0]], axis=1)
        biasTb = bias[b, 0].T  # [keys, queries]
        ebp = np.exp(
            np.concatenate(
                [biasTb[i0:, i0 : i0 + QL], biasTb[:i0, i0 : i0 + QL]], axis=0
            )
        )
        # [2048 keys, 1024 q] -> [128, (ic, jc, 512)] partition-contiguous
        ebr = (
            ebp.reshape(NJ, P, NI, 512)
            .transpose(1, 2, 0, 3)
            .reshape(P, NI * NJ * 512)
        )
        wq = w_qkv[0:CQ][esl] * (1.0 / np.sqrt(D))
        wk = w_qkv[CQ : 2 * CQ][esl]
        wv = w_qkv[2 * CQ : 3 * CQ][esl]
        wg = w_g[esl]
        wTc = np.concatenate([wq.T, wk.T, wv.T, wg.T], axis=1)  # [512, 1024]
        woTc = w_o[:, esl].T  # [256, 512] pair-major rows
        bgc = (0.5 * b_g[esl]).reshape(EL, 1)
        in_maps.append(
            {
                "xt_in": np.ascontiguousarray(_chunk128(xTp, CC)).astype(BF),
                "eb_in": np.ascontiguousarray(ebr).astype(BF),
                "wt_in": np.ascontiguousarray(_chunk128(wTc, CC)).astype(BF),
                "wot_in": np.ascontiguousarray(_chunk128(woTc, EC)).astype(BF),
                "bg_in": np.ascontiguousarray(bgc, np.float32),
                "bsel_in": _BSEL,
            }
        )

    nc = _get_nc()
    res = run_bass_kernel_spmd(nc, in_maps, core_ids=list(range(8)))
    LAST_RESULTS = res

    out = np.zeros((B, Q, CQ), np.float32)
    for core in range(8):
        b, qh = core >> 2, (core >> 1) & 1
        i0 = qh * QL
        out[b, i0 : i0 + QL] += res.results[core]["out"]
    out += b_o
    return out


# revision 13
# speedup vs baseline: 1.1440x; 1.1440x over previous
"""Trainium2 Bass kernel for nn_Attention_73289321939579.

Gated attention block (AlphaFold-style):
  qkv = q_x @ w_qkv.T ; q /= sqrt(64)
  scores = q k^T + bias ; attn = softmax(scores, keys)
  o = (attn @ v) * sigmoid(q_x @ w_g.T + b_g)
  out = o @ w_o.T + b_o

Sharding over 8 cores: core = b*4 + qh*2 + hq
  b  = batch (2)            -> data parallel
  qh = query half (2x1024)  -> bias/q sliced, output row-sliced
  hq = head quad (2x4 heads)-> tensor parallel; partial outputs summed on host

Device layout (per core, contractions on the SBUF partition axis; ALL bulk
inputs pre-arranged on host to [128 partitions, contiguous-lines] so each
is ONE DMA with 128 fat descriptors instead of thousands of 1KB ones):
  xt  [128, 4*2048]   bf16 = q_x[b].T key-rolled, channel-chunked
  eb  [128, 2*16*512] bf16 = exp(bias[b,0]).T chunked [key128, ic, jc, q];
        softmax computed as exp(qk) * exp(bias), exact in fp32/bf16
  wt  [128, 4*1024]   bf16 = [wq.T/8 | wk.T | wv.T | wg.T] channel-chunked
  wot [128, 2*512]    bf16 = w_o[:, heads].T pair-major
  bg  [256, 1] f32 = 0.5*b_g[heads]  (gate via 0.5*tanh(0.5x+0.5bg)+0.5;
        Tanh shares the ACT "exp" table set -> no table swap)
  bsel [128,128] f32 = 0/1: row 64 -> out partitions 0:64, row 65 -> 64:128
        (engine writes at partition base 65 are illegal -> host constant)

Schedule. ACT exp of the scores ([128,1024] per step at ~1.0us, no dtype
speedup on ACT) is the hard floor: 64 steps ~= 67us. Everything else hides
under it or under the PE:
  - bf16 operands everywhere: halves DMA/SBUF/LDWEIGHTS traffic, DVE
    multiply runs in 2x mode (~0.69us), PE cycles unchanged
  - one flat software-pipelined stream of 64 (ic,hp,jc) steps: per step
    S-pair matmuls -> exp -> eb-multiply, with the O-accumulate matmuls
    emitted 2 steps behind so the in-order PE never waits on the exp
    pipeline; no pass-boundary stalls
  - PSUM = 2x 2-bank S slots + 2x 1-bank O accumulators + 2x 1-bank
    broadcast/out-proj slots = exactly 8 banks
  - projections: a minimal upfront set (K pair0, Q ic0 pair0, V j0/j1,
    gate) runs before the first step; the remaining ~21 groups ride a
    closure queue, interleaved 2-per-step into the early steps' PE slack
  - each pass's normalize + gate + out-projection is also carried as
    closures into the following steps; denominator row moves ride
    SBUF->SBUF DMAs (partition-base-65 writes are DMA-only anyway)
  - normalize is pair-wise: both heads' denominators assembled in one
    tile, one reciprocal, one fp32 bsel-matmul broadcast
All matmuls are plain 128x128 mode (mode switches drain the PE array).
"""

import sys

for _p in ("/opt/trn_rl_repo",):
    if _p not in sys.path:
        sys.path.insert(0, _p)

from collections import deque
from contextlib import ExitStack

import ml_dtypes
import numpy as np

import concourse.bass as bass  # noqa: F401
import concourse.mybir as mybir
import concourse.tile as tile
from concourse import bacc
from concourse.bass_utils import run_bass_kernel_spmd

# ---- problem dims (hardcoded per contest contract) ----
B, Q, CQ = 2, 2048, 512
H, D = 8, 64
P = 128
QL = 1024          # queries per core
EL = 256           # e-dims per core (4 heads x 64)
HL = 4             # heads per core
CC = CQ // P       # 4 contraction chunks over channels
EC = EL // P       # 2 head-pairs
NJ = Q // P        # 16 key chunks
NI = QL // 512     # 2 query chunks of 512

F32 = mybir.dt.float32
BF16 = mybir.dt.bfloat16
MUL = mybir.AluOpType.mult
ADD = mybir.AluOpType.add
EXP = mybir.ActivationFunctionType.Exp
TANH = mybir.ActivationFunctionType.Tanh

OFF_Q, OFF_K, OFF_V, OFF_G = 0, EL, 2 * EL, 3 * EL


def _emit(tc, xt, eb, wt, wot, bg, bsel, outp):
    nc = tc.nc

    with ExitStack() as ctx:
        const = ctx.enter_context(tc.tile_pool(name="const", bufs=1))
        esp = ctx.enter_context(tc.tile_pool(name="esp", bufs=3))
        ptp = ctx.enter_context(tc.tile_pool(name="ptp", bufs=4))
        workp = ctx.enter_context(tc.tile_pool(name="workp", bufs=2))
        psum = ctx.enter_context(tc.tile_pool(name="psum", bufs=2, space="PSUM"))

        # ---- constants ----
        ones_sb = const.tile([P, P], F32, name="ones_sb", tag="ones_sb")
        nc.vector.memset(ones_sb, 1.0)
        bsel_sb = const.tile([P, P], F32, name="bsel_sb", tag="bsel_sb")
        nc.sync.dma_start(bsel_sb, bsel)
        bg_sb = const.tile([P, EC], F32, name="bg_sb", tag="bg_sb")
        nc.sync.dma_start(bg_sb, bg.rearrange("(o p) u -> p (o u)", p=P))
        woT_sb = const.tile([P, EC, CQ], BF16, name="woT_sb", tag="woT_sb")
        nc.sync.dma_start(woT_sb, wot.rearrange("p (o c) -> p o c", o=EC))
        rec_sbs = []
        for ri in range(2):
            rcb = const.tile([P, 512], F32, name=f"rec_sb{ri}", tag=f"rec_sb{ri}")
            nc.vector.memset(rcb, 0.0)
            rec_sbs.append(rcb)

        # ---- bulk inputs: one fat DMA each (128 descriptors) ----
        wT_sb = const.tile([P, CC, 4 * EL], BF16, name="wT_sb", tag="wT_sb")
        nc.sync.dma_start(wT_sb, wt.rearrange("p (c n) -> p c n", c=CC))
        # xt j4-major: chunk j4 unblocks proj group j4 as soon as it lands
        xT_sb = const.tile([P, Q // 512, CC, 512], BF16, name="xT_sb", tag="xT_sb")
        xtr = xt.rearrange("p (j c q) -> p j c q", j=Q // 512, c=CC)
        for j4 in range(Q // 512):
            nc.sync.dma_start(xT_sb[:, j4, :, :], xtr[:, j4, :, :])
        # exp(bias), fully resident; SAME SP queue AFTER xt so the input
        # stream is strictly ordered (eb chunks land long before their step)
        ebt_sb = const.tile([P, NI, NJ, 512], BF16, name="ebt_sb", tag="ebt_sb")
        ebr = eb.rearrange("p (i j q) -> p i j q", i=NI, j=NJ)
        for ic in range(NI):
            for jq in range(4):
                nc.sync.dma_start(
                    ebt_sb[:, ic, jq * 4 : (jq + 1) * 4, :],
                    ebr[:, ic, jq * 4 : (jq + 1) * 4, :],
                )

        # ---- resident intermediates ----
        kT_sb = const.tile([P, EC, Q], BF16, name="kT_sb", tag="kT_sb")
        qTp_sb = const.tile([P, HL, QL], BF16, name="qTp_sb", tag="qTp_sb")
        nc.vector.memset(qTp_sb[64:128, 0::2, :], 0.0)
        nc.vector.memset(qTp_sb[0:64, 1::2, :], 0.0)
        gp_sb = const.tile([P, EC, QL], BF16, name="gp_sb", tag="gp_sb")
        og_sb = const.tile([P, EC, QL], BF16, name="og_sb", tag="og_sb")
        v_sb = const.tile([P, NJ, HL, D + 1], BF16, name="v_sb", tag="v_sb")
        nc.vector.tensor_copy(
            out=v_sb[:, :, :, D],
            in_=ones_sb[:, 0:64].rearrange("p (a b) -> p a b", a=NJ),
        )

        # ---- projection groups (each: 4 accum matmuls + drain) ----
        def proj_k(ec, j4):
            ps_k = psum.tile([P, 512], F32, tag="s", name="ps_k")
            for c in range(CC):
                nc.tensor.matmul(
                    ps_k,
                    wT_sb[:, c, OFF_K + ec * P : OFF_K + (ec + 1) * P],
                    xT_sb[:, j4, c, :],
                    start=(c == 0),
                    stop=(c == CC - 1),
                )
            nc.vector.tensor_copy(
                out=kT_sb[:, ec, j4 * 512 : (j4 + 1) * 512], in_=ps_k
            )

        def proj_q(ic, ec):
            ps_q = psum.tile([P, 512], F32, tag="s", name="ps_q")
            for c in range(CC):
                nc.tensor.matmul(
                    ps_q,
                    wT_sb[:, c, OFF_Q + ec * P : OFF_Q + (ec + 1) * P],
                    xT_sb[:, ic, c, :],
                    start=(c == 0),
                    stop=(c == CC - 1),
                )
            sl = slice(ic * 512, (ic + 1) * 512)
            nc.vector.tensor_copy(out=qTp_sb[0:64, 2 * ec, sl], in_=ps_q[0:64, :])
            nc.vector.tensor_copy(
                out=qTp_sb[64:128, 2 * ec + 1, sl], in_=ps_q[64:128, :]
            )

        def proj_v(jc):
            ps_v = psum.tile([P, 512], F32, tag="s", name="ps_v")
            for c in range(CC):
                nc.tensor.matmul(
                    ps_v[:, :EL],
                    xT_sb[:, jc // 4, c, (jc % 4) * P : (jc % 4 + 1) * P],
                    wT_sb[:, c, OFF_V : OFF_V + EL],
                    start=(c == 0),
                    stop=(c == CC - 1),
                )
            nc.vector.tensor_copy(
                out=v_sb[:, jc, :, 0:D],
                in_=ps_v[:, :EL].rearrange("p (h d) -> p h d", h=HL),
            )

        def proj_g(ec, ic):
            ps_g = psum.tile([P, 512], F32, tag="s", name="ps_g")
            for c in range(CC):
                nc.tensor.matmul(
                    ps_g,
                    wT_sb[:, c, OFF_G + ec * P : OFF_G + (ec + 1) * P],
                    xT_sb[:, ic, c, :],
                    start=(c == 0),
                    stop=(c == CC - 1),
                )
            nc.scalar.activation(
                gp_sb[:, ec, ic * 512 : (ic + 1) * 512],
                ps_g,
                TANH,
                bias=bg_sb[:, ec : ec + 1],
                scale=0.5,
            )

        # ---- phase 0: warmup burst (PE clock ramp) over the DMA head ----
        warm_ps = psum.tile([P, 2, 512], F32, tag="s", name="warm_ps")
        for wi in range(10):
            nc.tensor.matmul(
                warm_ps[:, 0, 0:P],
                ones_sb,
                ones_sb,
                start=(wi == 0),
                stop=(wi == 9),
            )
        warm_sb = workp.tile([P, P], F32, name="warm_sb", tag="warm")
        nc.vector.tensor_copy(out=warm_sb[:, 0:P], in_=warm_ps[:, 0, 0:P])

        # ---- phase 1: all projections upfront (PE-bound ~22us, fed by
        # the ordered chunk DMAs; ACT cannot absorb proj work during
        # attention anyway - the PE has only ~90ns/step slack there) ----
        for j4 in range(Q // 512):
            proj_k(0, j4)
        proj_q(0, 0)
        for jc in range(NJ):
            proj_v(jc)
        for j4 in range(Q // 512):
            proj_k(1, j4)
        proj_q(0, 1)
        proj_q(1, 0)
        proj_q(1, 1)
        proj_g(0, 0)
        proj_g(1, 0)
        proj_g(0, 1)
        proj_g(1, 1)
        nc.vector.tensor_scalar(gp_sb, gp_sb, 0.5, 0.5, MUL, ADD)

        # ---- phase 2: flat pipelined attention over 64 steps ----
        outr = outp.rearrange("(o p) c -> p o c", p=P)

        def make_tail(ic, hp, o_ps, parity):
            """Normalize + gate closures for one finished (ic,hp) pass."""
            isl = slice(ic * 512, (ic + 1) * 512)
            rec_sb = rec_sbs[parity]
            o_pair = workp.tile([P, 512], F32, name="o_pair", tag="opair")
            wk = workp.tile([P, 512], F32, name="wk", tag="wk")
            recf = workp.tile([P, 512], F32, name="recf", tag="recf")
            ocp = workp.tile([P, 512], BF16, name="ocp", tag="ocp")
            bc_ps = psum.tile([P, 512], F32, tag="op", name="bc_ps")
            ops = []
            # zero wk rows 64:96 (legal base-64 band) before the denom
            # writes so the reciprocal reads deterministic data
            ops.append(lambda: nc.vector.memset(wk[64:96, :], 0.0))
            # even head -> o_pair rows 0:64 (denom parked in row 64)
            ops.append(
                lambda: nc.vector.tensor_copy(out=o_pair[0:65, :], in_=o_ps[0])
            )
            # odd head -> wk rows 0:64, its denom in wk[64]
            ops.append(lambda: nc.vector.tensor_copy(out=wk[0:65, :], in_=o_ps[1]))

            def _moves():
                # same SP queue: the row read of o_pair[64] (even denom ->
                # wk[96]) completes before the block write overwrites it
                nc.sync.dma_start(wk[96:97, :], o_pair[64:65, :])
                nc.sync.dma_start(o_pair[64:128, :], wk[0:64, :])

            ops.append(_moves)
            ops.append(
                lambda: nc.vector.reciprocal_approx_fast(
                    out=recf[0:97, :], in_=wk[0:97, :]
                )
            )
            # same-partition DVE row copies (bases 64/96 are legal):
            # rec[64] = 1/denom_odd, rec[96] = 1/denom_even
            ops.append(
                lambda: nc.vector.tensor_copy(
                    out=rec_sb[64:65, :], in_=recf[64:65, :]
                )
            )
            ops.append(
                lambda: nc.vector.tensor_copy(
                    out=rec_sb[96:97, :], in_=recf[96:97, :]
                )
            )
            ops.append(
                lambda: nc.tensor.matmul(
                    bc_ps, bsel_sb, rec_sb, start=True, stop=True
                )
            )
            ops.append(lambda: nc.vector.tensor_tensor(ocp, bc_ps, o_pair, MUL))
            ops.append(
                lambda: nc.vector.tensor_tensor(
                    og_sb[:, hp, isl], ocp, gp_sb[:, hp, isl], MUL
                )
            )
            return ops

        def make_outproj(ic):
            """Out-projection + store closures for one query block."""
            ops = []
            for ip4 in range(4):
                ip = ic * 4 + ip4
                ps_o = psum.tile([P, 512], F32, tag="op", name="ps_o")
                out_sb = workp.tile([P, 512], F32, name="out_sb", tag="outsb")

                def _mm(ps_o=ps_o, ip=ip):
                    for ec in range(EC):
                        nc.tensor.matmul(
                            ps_o,
                            og_sb[:, ec, ip * P : (ip + 1) * P],
                            woT_sb[:, ec, :],
                            start=(ec == 0),
                            stop=(ec == EC - 1),
                        )

                def _st(ps_o=ps_o, out_sb=out_sb, ip=ip):
                    nc.vector.tensor_copy(out=out_sb, in_=ps_o)
                    nc.sync.dma_start(outr[:, ip, :], out_sb)

                ops.append(_mm)
                ops.append(_st)
            return ops

        tailq = deque()
        PASSES = [(0, 0), (0, 1), (1, 0), (1, 1)]
        steps = [(ic, hp, jc) for (ic, hp) in PASSES for jc in range(NJ)]
        DEPTH = 2
        o_ps_by_pass = {}
        pts = {}

        def emit_O(fs):
            ic, hp, jc = steps[fs]
            o_ps = o_ps_by_pass[(ic, hp)]
            pt = pts.pop(fs)
            for hh in range(2):
                nc.tensor.matmul(
                    o_ps[hh],
                    v_sb[:, jc, 2 * hp + hh, :],
                    pt[:, hh, :],
                    start=(jc == 0),
                    stop=(jc == NJ - 1),
                )
            if jc == NJ - 1:
                tailq.extend(make_tail(ic, hp, o_ps, parity=(2 * ic + hp) % 2))
                if hp == 1:
                    tailq.extend(make_outproj(ic))

        for fs, (ic, hp, jc) in enumerate(steps):
            if jc == 0:
                o_ps_by_pass[(ic, hp)] = [
                    psum.tile(
                        [D + 1, 512], F32, tag="o", name=f"o_ps{hh}", bufs=2
                    )
                    for hh in range(2)
                ]
            isl = slice(ic * 512, (ic + 1) * 512)
            s_ps = psum.tile([P, 2, 512], F32, tag="s", name="s_ps")
            for hh in range(2):
                nc.tensor.matmul(
                    s_ps[:, hh, :],
                    kT_sb[:, hp, jc * P : (jc + 1) * P],
                    qTp_sb[:, 2 * hp + hh, isl],
                    start=True,
                    stop=True,
                )
            es = esp.tile([P, 2, 512], BF16, name="es", tag="es")
            nc.scalar.activation(es, s_ps, EXP)
            pt = ptp.tile([P, 2, 512], BF16, name="pt", tag="pt")
            ebb = ebt_sb[:, ic, jc, :][:, None, :].to_broadcast([P, 2, 512])
            nc.vector.tensor_tensor(pt, es, ebb, MUL)
            pts[fs] = pt
            for _ in range(2):
                if tailq:
                    tailq.popleft()()
            if fs >= DEPTH:
                emit_O(fs - DEPTH)
        for fs in range(len(steps) - DEPTH, len(steps)):
            emit_O(fs)
        while tailq:
            tailq.popleft()()


_CACHE = {}


def _get_nc():
    if "nc" not in _CACHE:
        nc = bacc.Bacc("TRN2", debug=False, enable_asserts=False)
        xt = nc.dram_tensor("xt_in", [P, CC * Q], BF16, kind="ExternalInput").ap()
        eb = nc.dram_tensor(
            "eb_in", [P, NI * NJ * 512], BF16, kind="ExternalInput"
        ).ap()
        wt = nc.dram_tensor("wt_in", [P, CC * 1024], BF16, kind="ExternalInput").ap()
        wot = nc.dram_tensor("wot_in", [P, EC * CQ], BF16, kind="ExternalInput").ap()
        bg = nc.dram_tensor("bg_in", [EL, 1], F32, kind="ExternalInput").ap()
        bsel = nc.dram_tensor("bsel_in", [P, P], F32, kind="ExternalInput").ap()
        outp = nc.dram_tensor("out", [QL, CQ], F32, kind="ExternalOutput").ap()
        with tile.TileContext(nc) as tc:
            _emit(tc, xt, eb, wt, wot, bg, bsel, outp)
        nc.compile()
        _CACHE["nc"] = nc
    return _CACHE["nc"]


LAST_RESULTS = None
BF = ml_dtypes.bfloat16
_BSEL = np.zeros((P, P), np.float32)
_BSEL[64, 64:128] = 1.0   # 1/denom_odd -> odd head rows
_BSEL[96, 0:64] = 1.0     # 1/denom_even -> even head rows


def _chunk128(a, nchunk):
    """[nchunk*128, L] -> [128, nchunk*L] partition-contiguous layout."""
    n, L = a.shape
    assert n == nchunk * P
    return a.reshape(nchunk, P, L).transpose(1, 0, 2).reshape(P, nchunk * L)


def kernel(q_x, kv_x, bias, w_qkv, w_o, b_o, w_g, b_g):
    global LAST_RESULTS
    q_x = np.asarray(q_x, np.float32)
    bias = np.asarray(bias, np.float32)
    w_qkv = np.asarray(w_qkv, np.float32)
    w_o = np.asarray(w_o, np.float32)
    b_o = np.asarray(b_o, np.float32)
    w_g = np.asarray(w_g, np.float32)
    b_g = np.asarray(b_g, np.float32)

    in_maps = []
    for core in range(8):
        b, qh, hq = core >> 2, (core >> 1) & 1, core & 1
        i0 = qh * QL
        esl = slice(hq * EL, (hq + 1) * EL)
        xTb = q_x[b].T  # [512, 2048]
        # roll keys so this core's queries are columns 0:QL
        xTp = np.concatenate([xTb[:, i0:], xTb[:, :i0]], axis=1)
        biasTb = bias[b, 0].T  # [keys, queries]
        ebp = np.exp(
            np.concatenate(
                [biasTb[i0:, i0 : i0 + QL], biasTb[:i0, i0 : i0 + QL]], axis=0
            )
        )
        # [2048 keys, 1024 q] -> [128, (ic, jc, 512)] partition-contiguous
        ebr = (
            ebp.reshape(NJ, P, NI, 512)
            .transpose(1, 2, 0, 3)
            .reshape(P, NI * NJ * 512)
        )
        wq = w_qkv[0:CQ][esl] * (1.0 / np.sqrt(D))
        wk = w_qkv[CQ : 2 * CQ][esl]
        wv = w_qkv[2 * CQ : 3 * CQ][esl]
        wg = w_g[esl]
        wTc = np.concatenate([wq.T, wk.T, wv.T, wg.T], axis=1)  # [512, 1024]
        woTc = w_o[:, esl].T  # [256, 512] pair-major rows
        bgc = (0.5 * b_g[esl]).reshape(EL, 1)
        in_maps.append(
            {
                "xt_in": np.ascontiguousarray(
                    xTp.reshape(CC, P, 4, 512)
                    .transpose(1, 2, 0, 3)
                    .reshape(P, CC * Q)
                ).astype(BF),
                "eb_in": np.ascontiguousarray(ebr).astype(BF),
                "wt_in": np.ascontiguousarray(_chunk128(wTc, CC)).astype(BF),
                "wot_in": np.ascontiguousarray(_chunk128(woTc, EC)).astype(BF),
                "bg_in": np.ascontiguousarray(bgc, np.float32),
                "bsel_in": _BSEL,
            }
        )

    nc = _get_nc()
    res = run_bass_kernel_spmd(nc, in_maps, core_ids=list(range(8)))
    LAST_RESULTS = res

    out = np.zeros((B, Q, CQ), np.float32)
    for core in range(8):
        b, qh = core >> 2, (core >> 1) & 1
        i0 = qh * QL
        out[b, i0 : i0 + QL] += res.results[core]["out"]
    out += b_o
    return out


# revision 14
# speedup vs baseline: 1.1729x; 1.0252x over previous
"""Trainium2 Bass kernel for nn_Attention_73289321939579.

Gated attention block (AlphaFold-style):
  qkv = q_x @ w_qkv.T ; q /= sqrt(64)
  scores = q k^T + bias ; attn = softmax(scores, keys)
  o = (attn @ v) * sigmoid(q_x @ w_g.T + b_g)
  out = o @ w_o.T + b_o

Sharding over 8 cores: core = b*4 + qh*2 + hq
  b  = batch (2)            -> data parallel
  qh = query half (2x1024)  -> bias/q sliced, output row-sliced
  hq = head quad (2x4 heads)-> tensor parallel; partial outputs summed on host

Device layout (per core, contractions on the SBUF partition axis; ALL bulk
inputs pre-arranged on host to [128 partitions, contiguous-lines] so each
is ONE DMA with 128 fat descriptors instead of thousands of 1KB ones):
  xt  [128, 4*2048]   bf16 = q_x[b].T key-rolled, channel-chunked
  eb  [128, 2*16*512] bf16 = exp(bias[b,0]).T chunked [key128, ic, jc, q];
        softmax computed as exp(qk) * exp(bias), exact in fp32/bf16
  wt  [128, 4*1024]   bf16 = [wq.T/8 | wk.T | wv.T | wg.T] channel-chunked
  wot [128, 2*512]    bf16 = w_o[:, heads].T pair-major
  bg  [256, 1] f32 = 0.5*b_g[heads]  (gate via 0.5*tanh(0.5x+0.5bg)+0.5;
        Tanh shares the ACT "exp" table set -> no table swap)
  bsel [128,128] f32 = 0/1: row 64 -> out partitions 0:64, row 65 -> 64:128
        (engine writes at partition base 65 are illegal -> host constant)

Schedule. ACT exp of the scores ([128,1024] per step at ~1.0us, no dtype
speedup on ACT) is the hard floor: 64 steps ~= 67us. Everything else hides
under it or under the PE:
  - bf16 operands everywhere: halves DMA/SBUF/LDWEIGHTS traffic, DVE
    multiply runs in 2x mode (~0.69us), PE cycles unchanged
  - one flat software-pipelined stream of 64 (ic,hp,jc) steps: per step
    S-pair matmuls -> exp -> eb-multiply, with the O-accumulate matmuls
    emitted 2 steps behind so the in-order PE never waits on the exp
    pipeline; no pass-boundary stalls
  - PSUM = 2x 2-bank S slots + 2x 1-bank O accumulators + 2x 1-bank
    broadcast/out-proj slots = exactly 8 banks
  - projections: a minimal upfront set (K pair0, Q ic0 pair0, V j0/j1,
    gate) runs before the first step; the remaining ~21 groups ride a
    closure queue, interleaved 2-per-step into the early steps' PE slack
  - each pass's normalize + gate + out-projection is also carried as
    closures into the following steps; denominator row moves ride
    SBUF->SBUF DMAs (partition-base-65 writes are DMA-only anyway)
  - normalize is pair-wise: both heads' denominators assembled in one
    tile, one reciprocal, one fp32 bsel-matmul broadcast
All matmuls are plain 128x128 mode (mode switches drain the PE array).
"""

import sys

for _p in ("/opt/trn_rl_repo",):
    if _p not in sys.path:
        sys.path.insert(0, _p)

from collections import deque
from contextlib import ExitStack

import ml_dtypes
import numpy as np

import concourse.bass as bass  # noqa: F401
import concourse.mybir as mybir
import concourse.tile as tile
from concourse import bacc
from concourse.bass_utils import run_bass_kernel_spmd

# ---- problem dims (hardcoded per contest contract) ----
B, Q, CQ = 2, 2048, 512
H, D = 8, 64
P = 128
QL = 1024          # queries per core
EL = 256           # e-dims per core (4 heads x 64)
HL = 4             # heads per core
CC = CQ // P       # 4 contraction chunks over channels
EC = EL // P       # 2 head-pairs
NJ = Q // P        # 16 key chunks
NI = QL // 512     # 2 query chunks of 512

F32 = mybir.dt.float32
BF16 = mybir.dt.bfloat16
MUL = mybir.AluOpType.mult
ADD = mybir.AluOpType.add
EXP = mybir.ActivationFunctionType.Exp
TANH = mybir.ActivationFunctionType.Tanh

OFF_Q, OFF_K, OFF_V, OFF_G = 0, EL, 2 * EL, 3 * EL


def _emit(tc, xt, eb, wt, wot, bg, bsel, outp):
    nc = tc.nc

    with ExitStack() as ctx:
        const = ctx.enter_context(tc.tile_pool(name="const", bufs=1))
        esp = ctx.enter_context(tc.tile_pool(name="esp", bufs=3))
        ptp = ctx.enter_context(tc.tile_pool(name="ptp", bufs=4))
        workp = ctx.enter_context(tc.tile_pool(name="workp", bufs=2))
        psum = ctx.enter_context(tc.tile_pool(name="psum", bufs=2, space="PSUM"))

        # ---- constants ----
        ones_sb = const.tile([P, P], F32, name="ones_sb", tag="ones_sb")
        nc.vector.memset(ones_sb, 1.0)
        bsel_sb = const.tile([P, P], F32, name="bsel_sb", tag="bsel_sb")
        nc.sync.dma_start(bsel_sb, bsel)
        bg_sb = const.tile([P, EC], F32, name="bg_sb", tag="bg_sb")
        nc.sync.dma_start(bg_sb, bg.rearrange("(o p) u -> p (o u)", p=P))
        woT_sb = const.tile([P, EC, CQ], BF16, name="woT_sb", tag="woT_sb")
        nc.sync.dma_start(woT_sb, wot.rearrange("p (o c) -> p o c", o=EC))
        rec_sbs = []
        for ri in range(2):
            rcb = const.tile([P, 512], F32, name=f"rec_sb{ri}", tag=f"rec_sb{ri}")
            nc.vector.memset(rcb, 0.0)
            rec_sbs.append(rcb)

        # ---- bulk inputs: one fat DMA each (128 descriptors) ----
        wT_sb = const.tile([P, CC, 4 * EL], BF16, name="wT_sb", tag="wT_sb")
        nc.sync.dma_start(wT_sb, wt.rearrange("p (c n) -> p c n", c=CC))
        # xt j4-major: chunk j4 unblocks proj group j4 as soon as it lands
        xT_sb = const.tile([P, Q // 512, CC, 512], BF16, name="xT_sb", tag="xT_sb")
        xtr = xt.rearrange("p (j c q) -> p j c q", j=Q // 512, c=CC)
        for j4 in range(Q // 512):
            nc.sync.dma_start(xT_sb[:, j4, :, :], xtr[:, j4, :, :])
        # exp(bias), fully resident; SAME SP queue AFTER xt so the input
        # stream is strictly ordered (eb chunks land long before their step)
        ebt_sb = const.tile([P, NI, NJ, 512], BF16, name="ebt_sb", tag="ebt_sb")
        ebr = eb.rearrange("p (i j q) -> p i j q", i=NI, j=NJ)
        for ic in range(NI):
            for jq in range(4):
                nc.sync.dma_start(
                    ebt_sb[:, ic, jq * 4 : (jq + 1) * 4, :],
                    ebr[:, ic, jq * 4 : (jq + 1) * 4, :],
                )

        # ---- resident intermediates ----
        kT_sb = const.tile([P, EC, Q], BF16, name="kT_sb", tag="kT_sb")
        qTp_sb = const.tile([P, HL, QL], BF16, name="qTp_sb", tag="qTp_sb")
        nc.vector.memset(qTp_sb[64:128, 0::2, :], 0.0)
        nc.vector.memset(qTp_sb[0:64, 1::2, :], 0.0)
        gp_sb = const.tile([P, EC, QL], BF16, name="gp_sb", tag="gp_sb")
        og_sb = const.tile([P, EC, QL], BF16, name="og_sb", tag="og_sb")
        v_sb = const.tile([P, NJ, HL, D + 1], BF16, name="v_sb", tag="v_sb")
        nc.vector.tensor_copy(
            out=v_sb[:, :, :, D],
            in_=ones_sb[:, 0:64].rearrange("p (a b) -> p a b", a=NJ),
        )

        # ---- projection groups (each: 4 accum matmuls + drain) ----
        # rotate across all three psum tags: during the projection phase the
        # attention accumulator banks are idle, and a 6-slot rotation hides
        # the drain-semaphore latency that a 2-slot one exposes
        _ptag = {"n": 0}

        def _proj_ps():
            t = ("s", "op", "o")[_ptag["n"] % 3]
            _ptag["n"] += 1
            return psum.tile([P, 512], F32, tag=t, name="ps_proj")

        def proj_k(ec, j4):
            ps_k = _proj_ps()
            for c in range(CC):
                nc.tensor.matmul(
                    ps_k,
                    wT_sb[:, c, OFF_K + ec * P : OFF_K + (ec + 1) * P],
                    xT_sb[:, j4, c, :],
                    start=(c == 0),
                    stop=(c == CC - 1),
                )
            nc.vector.tensor_copy(
                out=kT_sb[:, ec, j4 * 512 : (j4 + 1) * 512], in_=ps_k
            )

        def proj_q(ic, ec):
            ps_q = _proj_ps()
            for c in range(CC):
                nc.tensor.matmul(
                    ps_q,
                    wT_sb[:, c, OFF_Q + ec * P : OFF_Q + (ec + 1) * P],
                    xT_sb[:, ic, c, :],
                    start=(c == 0),
                    stop=(c == CC - 1),
                )
            sl = slice(ic * 512, (ic + 1) * 512)
            nc.vector.tensor_copy(out=qTp_sb[0:64, 2 * ec, sl], in_=ps_q[0:64, :])
            nc.vector.tensor_copy(
                out=qTp_sb[64:128, 2 * ec + 1, sl], in_=ps_q[64:128, :]
            )

        def proj_v(jc):
            ps_v = _proj_ps()
            for c in range(CC):
                nc.tensor.matmul(
                    ps_v[:, :EL],
                    xT_sb[:, jc // 4, c, (jc % 4) * P : (jc % 4 + 1) * P],
                    wT_sb[:, c, OFF_V : OFF_V + EL],
                    start=(c == 0),
                    stop=(c == CC - 1),
                )
            nc.vector.tensor_copy(
                out=v_sb[:, jc, :, 0:D],
                in_=ps_v[:, :EL].rearrange("p (h d) -> p h d", h=HL),
            )

        def proj_g(ec, ic):
            ps_g = _proj_ps()
            for c in range(CC):
                nc.tensor.matmul(
                    ps_g,
                    wT_sb[:, c, OFF_G + ec * P : OFF_G + (ec + 1) * P],
                    xT_sb[:, ic, c, :],
                    start=(c == 0),
                    stop=(c == CC - 1),
                )
            nc.scalar.activation(
                gp_sb[:, ec, ic * 512 : (ic + 1) * 512],
                ps_g,
                TANH,
                bias=bg_sb[:, ec : ec + 1],
                scale=0.5,
            )

        # ---- phase 0: warmup burst (PE clock ramp) over the DMA head ----
        warm_ps = psum.tile([P, 2, 512], F32, tag="s", name="warm_ps")
        for wi in range(10):
            nc.tensor.matmul(
                warm_ps[:, 0, 0:P],
                ones_sb,
                ones_sb,
                start=(wi == 0),
                stop=(wi == 9),
            )
        warm_sb = workp.tile([P, P], F32, name="warm_sb", tag="warm")
        nc.vector.tensor_copy(out=warm_sb[:, 0:P], in_=warm_ps[:, 0, 0:P])

        # ---- phase 1: all projections upfront (PE-bound ~22us, fed by
        # the ordered chunk DMAs; ACT cannot absorb proj work during
        # attention anyway - the PE has only ~90ns/step slack there) ----
        proj_k(0, 0)
        proj_q(0, 0)
        for jc in range(4):
            proj_v(jc)
        proj_k(0, 1)
        for jc in range(4, 8):
            proj_v(jc)
        proj_k(0, 2)
        for jc in range(8, 12):
            proj_v(jc)
        proj_k(0, 3)
        for jc in range(12, 16):
            proj_v(jc)
        for j4 in range(Q // 512):
            proj_k(1, j4)
        proj_q(0, 1)
        proj_q(1, 0)
        proj_q(1, 1)
        proj_g(0, 0)
        proj_g(1, 0)
        proj_g(0, 1)
        proj_g(1, 1)
        nc.vector.tensor_scalar(gp_sb, gp_sb, 0.5, 0.5, MUL, ADD)

        # ---- phase 2: flat pipelined attention over 64 steps ----
        outr = outp.rearrange("(o p) c -> p o c", p=P)

        def make_tail(ic, hp, o_ps, parity):
            """Normalize + gate closures for one finished (ic,hp) pass."""
            isl = slice(ic * 512, (ic + 1) * 512)
            rec_sb = rec_sbs[parity]
            o_pair = workp.tile([P, 512], F32, name="o_pair", tag="opair")
            wk = workp.tile([P, 512], F32, name="wk", tag="wk")
            recf = workp.tile([P, 512], F32, name="recf", tag="recf")
            ocp = workp.tile([P, 512], BF16, name="ocp", tag="ocp")
            bc_ps = psum.tile([P, 512], F32, tag="op", name="bc_ps")
            ops = []
            # zero wk rows 64:96 (legal base-64 band) before the denom
            # writes so the reciprocal reads deterministic data
            ops.append(lambda: nc.vector.memset(wk[64:96, :], 0.0))
            # even head -> o_pair rows 0:64 (denom parked in row 64)
            ops.append(
                lambda: nc.vector.tensor_copy(out=o_pair[0:65, :], in_=o_ps[0])
            )
            # odd head -> wk rows 0:64, its denom in wk[64]
            ops.append(lambda: nc.vector.tensor_copy(out=wk[0:65, :], in_=o_ps[1]))

            def _moves():
                # same SP queue: the row read of o_pair[64] (even denom ->
                # wk[96]) completes before the block write overwrites it
                nc.sync.dma_start(wk[96:97, :], o_pair[64:65, :])
                nc.sync.dma_start(o_pair[64:128, :], wk[0:64, :])

            ops.append(_moves)
            ops.append(
                lambda: nc.vector.reciprocal_approx_fast(
                    out=recf[0:97, :], in_=wk[0:97, :]
                )
            )
            # same-partition DVE row copies (bases 64/96 are legal):
            # rec[64] = 1/denom_odd, rec[96] = 1/denom_even
            ops.append(
                lambda: nc.vector.tensor_copy(
                    out=rec_sb[64:65, :], in_=recf[64:65, :]
                )
            )
            ops.append(
                lambda: nc.vector.tensor_copy(
                    out=rec_sb[96:97, :], in_=recf[96:97, :]
                )
            )
            ops.append(
                lambda: nc.tensor.matmul(
                    bc_ps, bsel_sb, rec_sb, start=True, stop=True
                )
            )
            ops.append(lambda: nc.vector.tensor_tensor(ocp, bc_ps, o_pair, MUL))
            ops.append(
                lambda: nc.vector.tensor_tensor(
                    og_sb[:, hp, isl], ocp, gp_sb[:, hp, isl], MUL
                )
            )
            return ops

        def make_outproj(ic):
            """Out-projection + store closures for one query block.
            All 4 chunks land in one tile, shipped by ONE DMA (four
            separate DMAs each paid multi-us semaphore gaps)."""
            ops = []
            out_ic = workp.tile([P, 4, 512], F32, name="out_ic", tag="outic")
            for ip4 in range(4):
                ip = ic * 4 + ip4
                ps_o = psum.tile([P, 512], F32, tag="op", name="ps_o")

                def _mm(ps_o=ps_o, ip=ip):
                    for ec in range(EC):
                        nc.tensor.matmul(
                            ps_o,
                            og_sb[:, ec, ip * P : (ip + 1) * P],
                            woT_sb[:, ec, :],
                            start=(ec == 0),
                            stop=(ec == EC - 1),
                        )

                def _st(ps_o=ps_o, ip4=ip4):
                    nc.vector.tensor_copy(out=out_ic[:, ip4, :], in_=ps_o)

                ops.append(_mm)
                ops.append(_st)
            ops.append(
                lambda: nc.sync.dma_start(
                    outr[:, ic * 4 : (ic + 1) * 4, :], out_ic
                )
            )
            return ops

        tailq = deque()
        PASSES = [(0, 0), (0, 1), (1, 0), (1, 1)]
        steps = [(ic, hp, jc) for (ic, hp) in PASSES for jc in range(NJ)]
        DEPTH = 2
        o_ps_by_pass = {}
        pts = {}

        def emit_O(fs):
            ic, hp, jc = steps[fs]
            o_ps = o_ps_by_pass[(ic, hp)]
            pt = pts.pop(fs)
            for hh in range(2):
                nc.tensor.matmul(
                    o_ps[hh],
                    v_sb[:, jc, 2 * hp + hh, :],
                    pt[:, hh, :],
                    start=(jc == 0),
                    stop=(jc == NJ - 1),
                )
            if jc == NJ - 1:
                tailq.extend(make_tail(ic, hp, o_ps, parity=(2 * ic + hp) % 2))
                if hp == 1:
                    tailq.extend(make_outproj(ic))

        for fs, (ic, hp, jc) in enumerate(steps):
            if jc == 0:
                o_ps_by_pass[(ic, hp)] = [
                    psum.tile(
                        [D + 1, 512], F32, tag="o", name=f"o_ps{hh}", bufs=2
                    )
                    for hh in range(2)
                ]
            isl = slice(ic * 512, (ic + 1) * 512)
            s_ps = psum.tile([P, 2, 512], F32, tag="s", name="s_ps")
            for hh in range(2):
                nc.tensor.matmul(
                    s_ps[:, hh, :],
                    kT_sb[:, hp, jc * P : (jc + 1) * P],
                    qTp_sb[:, 2 * hp + hh, isl],
                    start=True,
                    stop=True,
                )
            es = esp.tile([P, 2, 512], BF16, name="es", tag="es")
            nc.scalar.activation(es, s_ps, EXP)
            pt = ptp.tile([P, 2, 512], BF16, name="pt", tag="pt")
            ebb = ebt_sb[:, ic, jc, :][:, None, :].to_broadcast([P, 2, 512])
            nc.vector.tensor_tensor(pt, es, ebb, MUL)
            pts[fs] = pt
            for _ in range(2):
                if tailq:
                    tailq.popleft()()
            if fs >= DEPTH:
                emit_O(fs - DEPTH)
        for fs in range(len(steps) - DEPTH, len(steps)):
            emit_O(fs)
        # keep the PE clock from gating during the flush's dependency
        # latencies (HAM halves the clock after ~4us idle)
        warm2 = psum.tile([P, 2, 512], F32, tag="s", name="warm2")
        for wi in range(10):
            nc.tensor.matmul(
                warm2[:, 0, :],
                kT_sb[:, 0, 0:P],
                qTp_sb[:, 0, 0:512],
                start=(wi == 0),
                stop=(wi == 9),
            )
        while tailq:
            tailq.popleft()()


_CACHE = {}


def _get_nc():
    if "nc" not in _CACHE:
        nc = bacc.Bacc("TRN2", debug=False, enable_asserts=False)
        xt = nc.dram_tensor("xt_in", [P, CC * Q], BF16, kind="ExternalInput").ap()
        eb = nc.dram_tensor(
            "eb_in", [P, NI * NJ * 512], BF16, kind="ExternalInput"
        ).ap()
        wt = nc.dram_tensor("wt_in", [P, CC * 1024], BF16, kind="ExternalInput").ap()
        wot = nc.dram_tensor("wot_in", [P, EC * CQ], BF16, kind="ExternalInput").ap()
        bg = nc.dram_tensor("bg_in", [EL, 1], F32, kind="ExternalInput").ap()
        bsel = nc.dram_tensor("bsel_in", [P, P], F32, kind="ExternalInput").ap()
        outp = nc.dram_tensor("out", [QL, CQ], F32, kind="ExternalOutput").ap()
        with tile.TileContext(nc) as tc:
            _emit(tc, xt, eb, wt, wot, bg, bsel, outp)
        nc.compile()
        _CACHE["nc"] = nc
    return _CACHE["nc"]


LAST_RESULTS = None
BF = ml_dtypes.bfloat16
_BSEL = np.zeros((P, P), np.float32)
_BSEL[64, 64:128] = 1.0   # 1/denom_odd -> odd head rows
_BSEL[96, 0:64] = 1.0     # 1/denom_even -> even head rows


def _chunk128(a, nchunk):
    """[nchunk*128, L] -> [128, nchunk*L] partition-contiguous layout."""
    n, L = a.shape
    assert n == nchunk * P
    return a.reshape(nchunk, P, L).transpose(1, 0, 2).reshape(P, nchunk * L)


def kernel(q_x, kv_x, bias, w_qkv, w_o, b_o, w_g, b_g):
    global LAST_RESULTS
    q_x = np.asarray(q_x, np.float32)
    bias = np.asarray(bias, np.float32)
    w_qkv = np.asarray(w_qkv, np.float32)
    w_o = np.asarray(w_o, np.float32)
    b_o = np.asarray(b_o, np.float32)
    w_g = np.asarray(w_g, np.float32)
    b_g = np.asarray(b_g, np.float32)

    in_maps = []
    for core in range(8):
        b, qh, hq = core >> 2, (core >> 1) & 1, core & 1
        i0 = qh * QL
        esl = slice(hq * EL, (hq + 1) * EL)
        xTb = q_x[b].T  # [512, 2048]
        # roll keys so this core's queries are columns 0:QL
        xTp = np.concatenate([xTb[:, i0:], xTb[:, :i0]], axis=1)
        biasTb = bias[b, 0].T  # [keys, queries]
        ebp = np.exp(
            np.concatenate(
                [biasTb[i0:, i0 : i0 + QL], biasTb[:i0, i0 : i0 + QL]], axis=0
            )
        )
        # [2048 keys, 1024 q] -> [128, (ic, jc, 512)] partition-contiguous
        ebr = (
            ebp.reshape(NJ, P, NI, 512)
            .transpose(1, 2, 0, 3)
            .reshape(P, NI * NJ * 512)
        )
        wq = w_qkv[0:CQ][esl] * (1.0 / np.sqrt(D))
        wk = w_qkv[CQ : 2 * CQ][esl]
        wv = w_qkv[2 * CQ : 3 * CQ][esl]
        wg = w_g[esl]
        wTc = np.concatenate([wq.T, wk.T, wv.T, wg.T], axis=1)  # [512, 1024]
        woTc = w_o[:, esl].T  # [256, 512] pair-major rows
        bgc = (0.5 * b_g[esl]).reshape(EL, 1)
        in_maps.append(
            {
                "xt_in": np.ascontiguousarray(
                    xTp.reshape(CC, P, 4, 512)
                    .transpose(1, 2, 0, 3)
                    .reshape(P, CC * Q)
                ).astype(BF),
                "eb_in": np.ascontiguousarray(ebr).astype(BF),
                "wt_in": np.ascontiguousarray(_chunk128(wTc, CC)).astype(BF),
                "wot_in": np.ascontiguousarray(_chunk128(woTc, EC)).astype(BF),
                "bg_in": np.ascontiguousarray(bgc, np.float32),
                "bsel_in": _BSEL,
            }
        )

    nc = _get_nc()
    res = run_bass_kernel_spmd(nc, in_maps, core_ids=list(range(8)))
    LAST_RESULTS = res

    out = np.zeros((B, Q, CQ), np.float32)
    for core in range(8):
        b, qh = core >> 2, (core >> 1) & 1
        i0 = qh * QL
        out[b, i0 : i0 + QL] += res.results[core]["out"]
    out += b_o
    return out


# revision 15
# speedup vs baseline: 1.2248x; 1.0443x over previous
"""Trainium2 Bass kernel for nn_Attention_73289321939579.

Gated attention block (AlphaFold-style):
  qkv = q_x @ w_qkv.T ; q /= sqrt(64)
  scores = q k^T + bias ; attn = softmax(scores, keys)
  o = (attn @ v) * sigmoid(q_x @ w_g.T + b_g)
  out = o @ w_o.T + b_o

Sharding over 8 cores: core = b*4 + qh*2 + hq
  b  = batch (2)            -> data parallel
  qh = query half (2x1024)  -> bias/q sliced, output row-sliced
  hq = head quad (2x4 heads)-> tensor parallel; partial outputs summed on host

Device layout (per core, contractions on the SBUF partition axis; ALL bulk
inputs pre-arranged on host to [128 partitions, contiguous-lines] so each
is ONE DMA with 128 fat descriptors instead of thousands of 1KB ones):
  xt  [128, 4*2048]   bf16 = q_x[b].T key-rolled, channel-chunked
  eb  [128, 2*16*512] bf16 = exp(bias[b,0]).T chunked [key128, ic, jc, q];
        softmax computed as exp(qk) * exp(bias), exact in fp32/bf16
  wt  [128, 4*1024]   bf16 = [wq.T/8 | wk.T | wv.T | wg.T] channel-chunked
  wot [128, 2*512]    bf16 = w_o[:, heads].T pair-major
  bg  [256, 1] f32 = 0.5*b_g[heads]  (gate via 0.5*tanh(0.5x+0.5bg)+0.5;
        Tanh shares the ACT "exp" table set -> no table swap)
  bsel [128,128] f32 = 0/1: row 64 -> out partitions 0:64, row 65 -> 64:128
        (engine writes at partition base 65 are illegal -> host constant)

Schedule. ACT exp of the scores ([128,1024] per step at ~1.0us, no dtype
speedup on ACT) is the hard floor: 64 steps ~= 67us. Everything else hides
under it or under the PE:
  - bf16 operands everywhere: halves DMA/SBUF/LDWEIGHTS traffic, DVE
    multiply runs in 2x mode (~0.69us), PE cycles unchanged
  - one flat software-pipelined stream of 64 (ic,hp,jc) steps: per step
    S-pair matmuls -> exp -> eb-multiply, with the O-accumulate matmuls
    emitted 2 steps behind so the in-order PE never waits on the exp
    pipeline; no pass-boundary stalls
  - PSUM = 2x 2-bank S slots + 2x 1-bank O accumulators + 2x 1-bank
    broadcast/out-proj slots = exactly 8 banks
  - projections: a minimal upfront set (K pair0, Q ic0 pair0, V j0/j1,
    gate) runs before the first step; the remaining ~21 groups ride a
    closure queue, interleaved 2-per-step into the early steps' PE slack
  - each pass's normalize + gate + out-projection is also carried as
    closures into the following steps; denominator row moves ride
    SBUF->SBUF DMAs (partition-base-65 writes are DMA-only anyway)
  - normalize is pair-wise: both heads' denominators assembled in one
    tile, one reciprocal, one fp32 bsel-matmul broadcast
All matmuls are plain 128x128 mode (mode switches drain the PE array).
"""

import sys

for _p in ("/opt/trn_rl_repo",):
    if _p not in sys.path:
        sys.path.insert(0, _p)

from collections import deque
from contextlib import ExitStack

import ml_dtypes
import numpy as np

import concourse.bass as bass  # noqa: F401
import concourse.mybir as mybir
import concourse.tile as tile
from concourse import bacc
from concourse.bass_utils import run_bass_kernel_spmd

# ---- problem dims (hardcoded per contest contract) ----
B, Q, CQ = 2, 2048, 512
H, D = 8, 64
P = 128
QL = 1024          # queries per core
EL = 256           # e-dims per core (4 heads x 64)
HL = 4             # heads per core
CC = CQ // P       # 4 contraction chunks over channels
EC = EL // P       # 2 head-pairs
NJ = Q // P        # 16 key chunks
NI = QL // 512     # 2 query chunks of 512

F32 = mybir.dt.float32
BF16 = mybir.dt.bfloat16
MUL = mybir.AluOpType.mult
ADD = mybir.AluOpType.add
EXP = mybir.ActivationFunctionType.Exp
TANH = mybir.ActivationFunctionType.Tanh

OFF_Q, OFF_K, OFF_V, OFF_G = 0, EL, 2 * EL, 3 * EL


def _emit(tc, xt, eb, wt, wot, bg, bsel, outp):
    nc = tc.nc

    with ExitStack() as ctx:
        const = ctx.enter_context(tc.tile_pool(name="const", bufs=1))
        esp = ctx.enter_context(tc.tile_pool(name="esp", bufs=3))
        ptp = ctx.enter_context(tc.tile_pool(name="ptp", bufs=4))
        workp = ctx.enter_context(tc.tile_pool(name="workp", bufs=2))
        psum = ctx.enter_context(tc.tile_pool(name="psum", bufs=2, space="PSUM"))

        # ---- ALL input DMAs first: nothing may delay the SP queue (the
        # GPSIMD memsets' first launch costs ~3us and used to push the
        # whole input stream to t=7.7us) ----
        bsel_sb = const.tile([P, P], F32, name="bsel_sb", tag="bsel_sb")
        nc.sync.dma_start(bsel_sb, bsel)
        bg_sb = const.tile([P, EC], F32, name="bg_sb", tag="bg_sb")
        nc.sync.dma_start(bg_sb, bg.rearrange("(o p) u -> p (o u)", p=P))
        woT_sb = const.tile([P, EC, CQ], BF16, name="woT_sb", tag="woT_sb")
        nc.sync.dma_start(woT_sb, wot.rearrange("p (o c) -> p o c", o=EC))
        wT_sb = const.tile([P, CC, 4 * EL], BF16, name="wT_sb", tag="wT_sb")
        nc.sync.dma_start(wT_sb, wt.rearrange("p (c n) -> p c n", c=CC))
        # xt j4-major: chunk j4 unblocks proj group j4 as soon as it lands
        xT_sb = const.tile([P, Q // 512, CC, 512], BF16, name="xT_sb", tag="xT_sb")
        xtr = xt.rearrange("p (j c q) -> p j c q", j=Q // 512, c=CC)
        for j4 in range(Q // 512):
            nc.sync.dma_start(xT_sb[:, j4, :, :], xtr[:, j4, :, :])
        # exp(bias), fully resident; SAME SP queue AFTER xt so the input
        # stream is strictly ordered (eb chunks land long before their step)
        ebt_sb = const.tile([P, NI, NJ, 512], BF16, name="ebt_sb", tag="ebt_sb")
        ebr = eb.rearrange("p (i j q) -> p i j q", i=NI, j=NJ)
        for ic in range(NI):
            for jq in range(4):
                nc.sync.dma_start(
                    ebt_sb[:, ic, jq * 4 : (jq + 1) * 4, :],
                    ebr[:, ic, jq * 4 : (jq + 1) * 4, :],
                )

        # ---- resident intermediates + on-chip constants (GPSIMD memsets
        # run concurrently with the input stream) ----
        rec_sbs = []
        for ri in range(2):
            rcb = const.tile([P, 512], F32, name=f"rec_sb{ri}", tag=f"rec_sb{ri}")
            nc.vector.memset(rcb, 0.0)
            rec_sbs.append(rcb)
        kT_sb = const.tile([P, EC, Q], BF16, name="kT_sb", tag="kT_sb")
        qTp_sb = const.tile([P, HL, QL], BF16, name="qTp_sb", tag="qTp_sb")
        nc.vector.memset(qTp_sb[64:128, 0::2, :], 0.0)
        nc.vector.memset(qTp_sb[0:64, 1::2, :], 0.0)
        gp_sb = const.tile([P, EC, QL], BF16, name="gp_sb", tag="gp_sb")
        og_sb = const.tile([P, EC, QL], BF16, name="og_sb", tag="og_sb")
        v_sb = const.tile([P, NJ, HL, D + 1], BF16, name="v_sb", tag="v_sb")
        nc.vector.memset(v_sb[:, :, :, D], 1.0)

        # ---- projection groups (each: 4 accum matmuls + drain) ----
        # rotate across all three psum tags: during the projection phase the
        # attention accumulator banks are idle, and a 6-slot rotation hides
        # the drain-semaphore latency that a 2-slot one exposes
        _ptag = {"n": 0}

        def _proj_ps():
            t = ("s", "op", "o")[_ptag["n"] % 3]
            _ptag["n"] += 1
            return psum.tile([P, 512], F32, tag=t, name="ps_proj")

        def proj_k(ec, j4):
            ps_k = _proj_ps()
            for c in range(CC):
                nc.tensor.matmul(
                    ps_k,
                    wT_sb[:, c, OFF_K + ec * P : OFF_K + (ec + 1) * P],
                    xT_sb[:, j4, c, :],
                    start=(c == 0),
                    stop=(c == CC - 1),
                )
            nc.vector.tensor_copy(
                out=kT_sb[:, ec, j4 * 512 : (j4 + 1) * 512], in_=ps_k
            )

        def proj_q(ic, ec):
            ps_q = _proj_ps()
            for c in range(CC):
                nc.tensor.matmul(
                    ps_q,
                    wT_sb[:, c, OFF_Q + ec * P : OFF_Q + (ec + 1) * P],
                    xT_sb[:, ic, c, :],
                    start=(c == 0),
                    stop=(c == CC - 1),
                )
            sl = slice(ic * 512, (ic + 1) * 512)
            nc.vector.tensor_copy(out=qTp_sb[0:64, 2 * ec, sl], in_=ps_q[0:64, :])
            nc.vector.tensor_copy(
                out=qTp_sb[64:128, 2 * ec + 1, sl], in_=ps_q[64:128, :]
            )

        def proj_v(jc):
            ps_v = _proj_ps()
            for c in range(CC):
                nc.tensor.matmul(
                    ps_v[:, :EL],
                    xT_sb[:, jc // 4, c, (jc % 4) * P : (jc % 4 + 1) * P],
                    wT_sb[:, c, OFF_V : OFF_V + EL],
                    start=(c == 0),
                    stop=(c == CC - 1),
                )
            nc.vector.tensor_copy(
                out=v_sb[:, jc, :, 0:D],
                in_=ps_v[:, :EL].rearrange("p (h d) -> p h d", h=HL),
            )

        def proj_g(ec, ic):
            ps_g = _proj_ps()
            for c in range(CC):
                nc.tensor.matmul(
                    ps_g,
                    wT_sb[:, c, OFF_G + ec * P : OFF_G + (ec + 1) * P],
                    xT_sb[:, ic, c, :],
                    start=(c == 0),
                    stop=(c == CC - 1),
                )
            nc.scalar.activation(
                gp_sb[:, ec, ic * 512 : (ic + 1) * 512],
                ps_g,
                TANH,
                bias=bg_sb[:, ec : ec + 1],
                scale=0.5,
            )

        # ---- phase 0: warmup burst (PE clock ramp) over the DMA head ----
        warm_ps = psum.tile([P, 2, 512], F32, tag="s", name="warm_ps")
        for wi in range(24):
            nc.tensor.matmul(
                warm_ps[:, 0, 0:P],
                bsel_sb,
                bsel_sb,
                start=(wi == 0),
                stop=(wi == 23),
            )
        warm_sb = workp.tile([P, P], F32, name="warm_sb", tag="warm")
        nc.vector.tensor_copy(out=warm_sb[:, 0:P], in_=warm_ps[:, 0, 0:P])

        # ---- phase 1: all projections upfront (PE-bound ~22us, fed by
        # the ordered chunk DMAs; ACT cannot absorb proj work during
        # attention anyway - the PE has only ~90ns/step slack there) ----
        proj_k(0, 0)
        proj_q(0, 0)
        for jc in range(4):
            proj_v(jc)
        proj_k(0, 1)
        for jc in range(4, 8):
            proj_v(jc)
        proj_k(0, 2)
        for jc in range(8, 12):
            proj_v(jc)
        proj_k(0, 3)
        for jc in range(12, 16):
            proj_v(jc)
        for j4 in range(Q // 512):
            proj_k(1, j4)
        proj_q(0, 1)
        proj_q(1, 0)
        proj_q(1, 1)
        proj_g(0, 0)
        proj_g(1, 0)
        proj_g(0, 1)
        proj_g(1, 1)
        nc.vector.tensor_scalar(gp_sb, gp_sb, 0.5, 0.5, MUL, ADD)

        # ---- phase 2: flat pipelined attention over 64 steps ----
        outr = outp.rearrange("(o p) c -> p o c", p=P)

        def make_tail(ic, hp, o_ps, parity):
            """Normalize + gate closures for one finished (ic,hp) pass."""
            isl = slice(ic * 512, (ic + 1) * 512)
            rec_sb = rec_sbs[parity]
            o_pair = workp.tile([P, 512], F32, name="o_pair", tag="opair")
            wk = workp.tile([P, 512], F32, name="wk", tag="wk")
            recf = workp.tile([P, 512], F32, name="recf", tag="recf")
            ocp = workp.tile([P, 512], BF16, name="ocp", tag="ocp")
            bc_ps = psum.tile([P, 512], F32, tag="op", name="bc_ps")
            ops = []
            # zero wk rows 64:96 (legal base-64 band) before the denom
            # writes so the reciprocal reads deterministic data
            ops.append(lambda: nc.vector.memset(wk[64:96, :], 0.0))
            # even head -> o_pair rows 0:64 (denom parked in row 64)
            ops.append(
                lambda: nc.vector.tensor_copy(out=o_pair[0:65, :], in_=o_ps[0])
            )
            # odd head -> wk rows 0:64, its denom in wk[64]
            ops.append(lambda: nc.vector.tensor_copy(out=wk[0:65, :], in_=o_ps[1]))

            def _moves():
                # same SP queue: the row read of o_pair[64] (even denom ->
                # wk[96]) completes before the block write overwrites it
                nc.sync.dma_start(wk[96:97, :], o_pair[64:65, :])
                nc.sync.dma_start(o_pair[64:128, :], wk[0:64, :])

            ops.append(_moves)
            ops.append(
                lambda: nc.vector.reciprocal_approx_fast(
                    out=recf[0:97, :], in_=wk[0:97, :]
                )
            )
            # same-partition DVE row copies (bases 64/96 are legal):
            # rec[64] = 1/denom_odd, rec[96] = 1/denom_even
            ops.append(
                lambda: nc.vector.tensor_copy(
                    out=rec_sb[64:65, :], in_=recf[64:65, :]
                )
            )
            ops.append(
                lambda: nc.vector.tensor_copy(
                    out=rec_sb[96:97, :], in_=recf[96:97, :]
                )
            )
            ops.append(
                lambda: nc.tensor.matmul(
                    bc_ps, bsel_sb, rec_sb, start=True, stop=True
                )
            )
            ops.append(lambda: nc.vector.tensor_tensor(ocp, bc_ps, o_pair, MUL))
            ops.append(
                lambda: nc.vector.tensor_tensor(
                    og_sb[:, hp, isl], ocp, gp_sb[:, hp, isl], MUL
                )
            )
            return ops

        def make_outproj(ic):
            """Out-projection + store closures for one query block.
            All 4 chunks land in one tile, shipped by ONE DMA (four
            separate DMAs each paid multi-us semaphore gaps)."""
            ops = []
            out_ic = workp.tile([P, 4, 512], F32, name="out_ic", tag="outic")
            for ip4 in range(4):
                ip = ic * 4 + ip4
                ps_o = psum.tile([P, 512], F32, tag="op", name="ps_o")

                def _mm(ps_o=ps_o, ip=ip):
                    for ec in range(EC):
                        nc.tensor.matmul(
                            ps_o,
                            og_sb[:, ec, ip * P : (ip + 1) * P],
                            woT_sb[:, ec, :],
                            start=(ec == 0),
                            stop=(ec == EC - 1),
                        )

                def _st(ps_o=ps_o, ip4=ip4, ip=ip):
                    nc.vector.tensor_copy(out=out_ic[:, ip4, :], in_=ps_o)
                    # alternate queues: each 256KB chunk ships immediately
                    # instead of one tail-end 1MB DMA trickling for ~9us
                    eng = nc.sync if ip4 % 2 == 0 else nc.scalar
                    eng.dma_start(outr[:, ip, :], out_ic[:, ip4, :])

                ops.append(_mm)
                ops.append(_st)
            return ops

        tailq = deque()
        PASSES = [(0, 0), (0, 1), (1, 0), (1, 1)]
        steps = [(ic, hp, jc) for (ic, hp) in PASSES for jc in range(NJ)]
        DEPTH = 2
        o_ps_by_pass = {}
        pts = {}

        def emit_O(fs):
            ic, hp, jc = steps[fs]
            o_ps = o_ps_by_pass[(ic, hp)]
            pt = pts.pop(fs)
            for hh in range(2):
                nc.tensor.matmul(
                    o_ps[hh],
                    v_sb[:, jc, 2 * hp + hh, :],
                    pt[:, hh, :],
                    start=(jc == 0),
                    stop=(jc == NJ - 1),
                )
            if jc == NJ - 1:
                tailq.extend(make_tail(ic, hp, o_ps, parity=(2 * ic + hp) % 2))
                if hp == 1:
                    tailq.extend(make_outproj(ic))

        for fs, (ic, hp, jc) in enumerate(steps):
            if jc == 0:
                o_ps_by_pass[(ic, hp)] = [
                    psum.tile(
                        [D + 1, 512], F32, tag="o", name=f"o_ps{hh}", bufs=2
                    )
                    for hh in range(2)
                ]
            isl = slice(ic * 512, (ic + 1) * 512)
            s_ps = psum.tile([P, 2, 512], F32, tag="s", name="s_ps")
            for hh in range(2):
                nc.tensor.matmul(
                    s_ps[:, hh, :],
                    kT_sb[:, hp, jc * P : (jc + 1) * P],
                    qTp_sb[:, 2 * hp + hh, isl],
                    start=True,
                    stop=True,
                )
            es = esp.tile([P, 2, 512], BF16, name="es", tag="es")
            nc.scalar.activation(es, s_ps, EXP)
            pt = ptp.tile([P, 2, 512], BF16, name="pt", tag="pt")
            ebb = ebt_sb[:, ic, jc, :][:, None, :].to_broadcast([P, 2, 512])
            nc.vector.tensor_tensor(pt, es, ebb, MUL)
            pts[fs] = pt
            # 1 closure/step keeps DVE under the ACT ceiling; the first
            # steps of a pass take 2 so the o_ps drains land before O(0)
            nops = 2 if jc < 6 else 1
            for _ in range(nops):
                if tailq:
                    tailq.popleft()()
            if fs >= DEPTH:
                emit_O(fs - DEPTH)
        for fs in range(len(steps) - DEPTH, len(steps)):
            emit_O(fs)
        # keep the PE clock from gating during the flush's dependency
        # latencies (HAM halves the clock after ~4us idle)
        warm2 = psum.tile([P, 2, 512], F32, tag="s", name="warm2")
        for wi in range(16):
            nc.tensor.matmul(
                warm2[:, 0, :],
                kT_sb[:, 0, 0:P],
                qTp_sb[:, 0, 0:512],
                start=(wi == 0),
                stop=(wi == 15),
            )
        while tailq:
            tailq.popleft()()


_CACHE = {}


def _get_nc():
    if "nc" not in _CACHE:
        nc = bacc.Bacc("TRN2", debug=False, enable_asserts=False)
        xt = nc.dram_tensor("xt_in", [P, CC * Q], BF16, kind="ExternalInput").ap()
        eb = nc.dram_tensor(
            "eb_in", [P, NI * NJ * 512], BF16, kind="ExternalInput"
        ).ap()
        wt = nc.dram_tensor("wt_in", [P, CC * 1024], BF16, kind="ExternalInput").ap()
        wot = nc.dram_tensor("wot_in", [P, EC * CQ], BF16, kind="ExternalInput").ap()
        bg = nc.dram_tensor("bg_in", [EL, 1], F32, kind="ExternalInput").ap()
        bsel = nc.dram_tensor("bsel_in", [P, P], F32, kind="ExternalInput").ap()
        outp = nc.dram_tensor("out", [QL, CQ], F32, kind="ExternalOutput").ap()
        with tile.TileContext(nc) as tc:
            _emit(tc, xt, eb, wt, wot, bg, bsel, outp)
        nc.compile()
        _CACHE["nc"] = nc
    return _CACHE["nc"]


LAST_RESULTS = None
BF = ml_dtypes.bfloat16
_BSEL = np.zeros((P, P), np.float32)
_BSEL[64, 64:128] = 1.0   # 1/denom_odd -> odd head rows
_BSEL[96, 0:64] = 1.0     # 1/denom_even -> even head rows


def _chunk128(a, nchunk):
    """[nchunk*128, L] -> [128, nchunk*L] partition-contiguous layout."""
    n, L = a.shape
    assert n == nchunk * P
    return a.reshape(nchunk, P, L).transpose(1, 0, 2).reshape(P, nchunk * L)


def kernel(q_x, kv_x, bias, w_qkv, w_o, b_o, w_g, b_g):
    global LAST_RESULTS
    q_x = np.asarray(q_x, np.float32)
    bias = np.asarray(bias, np.float32)
    w_qkv = np.asarray(w_qkv, np.float32)
    w_o = np.asarray(w_o, np.float32)
    b_o = np.asarray(b_o, np.float32)
    w_g = np.asarray(w_g, np.float32)
    b_g = np.asarray(b_g, np.float32)

    in_maps = []
    for core in range(8):
        b, qh, hq = core >> 2, (core >> 1) & 1, core & 1
        i0 = qh * QL
        esl = slice(hq * EL, (hq + 1) * EL)
        xTb = q_x[b].T  # [512, 2048]
        # roll keys so this core's queries are columns 0:QL
        xTp = np.concatenate([xTb[:, i0:], xTb[:, :i0]], axis=1)
        biasTb = bias[b, 0].T  # [keys, queries]
        ebp = np.exp(
            np.concatenate(
                [biasTb[i0:, i0 : i0 + QL], biasTb[:i0, i0 : i0 + QL]], axis=0
            )
        )
        # [2048 keys, 1024 q] -> [128, (ic, jc, 512)] partition-contiguous
        ebr = (
            ebp.reshape(NJ, P, NI, 512)
            .transpose(1, 2, 0, 3)
            .reshape(P, NI * NJ * 512)
        )
        wq = w_qkv[0:CQ][esl] * (1.0 / np.sqrt(D))
        wk = w_qkv[CQ : 2 * CQ][esl]
        wv = w_qkv[2 * CQ : 3 * CQ][esl]
        wg = w_g[esl]
        wTc = np.concatenate([wq.T, wk.T, wv.T, wg.T], axis=1)  # [512, 1024]
        woTc = w_o[:, esl].T  # [256, 512] pair-major rows
        bgc = (0.5 * b_g[esl]).reshape(EL, 1)
        in_maps.append(
            {
                "xt_in": np.ascontiguousarray(
                    xTp.reshape(CC, P, 4, 512)
                    .transpose(1, 2, 0, 3)
                    .reshape(P, CC * Q)
                ).astype(BF),
                "eb_in": np.ascontiguousarray(ebr).astype(BF),
                "wt_in": np.ascontiguousarray(_chunk128(wTc, CC)).astype(BF),
                "wot_in": np.ascontiguousarray(_chunk128(woTc, EC)).astype(BF),
                "bg_in": np.ascontiguousarray(bgc, np.float32),
                "bsel_in": _BSEL,
            }
        )

    nc = _get_nc()
    res = run_bass_kernel_spmd(nc, in_maps, core_ids=list(range(8)))
    LAST_RESULTS = res

    out = np.zeros((B, Q, CQ), np.float32)
    for core in range(8):
        b, qh = core >> 2, (core >> 1) & 1
        i0 = qh * QL
        out[b, i0 : i0 + QL] += res.results[core]["out"]
    out += b_o
    return out


# revision 21
# speedup vs baseline: 1.2524x; 1.0225x over previous
"""Trainium2 Bass kernel for nn_Attention_73289321939579.

Gated attention block (AlphaFold-style):
  qkv = q_x @ w_qkv.T ; q /= sqrt(64)
  scores = q k^T + bias ; attn = softmax(scores, keys)
  o = (attn @ v) * sigmoid(q_x @ w_g.T + b_g)
  out = o @ w_o.T + b_o

Sharding over 8 cores: core = b*4 + qh*2 + hq
  b  = batch (2)            -> data parallel
  qh = query half (2x1024)  -> bias/q sliced, output row-sliced
  hq = head quad (2x4 heads)-> tensor parallel; partial outputs summed on host

Device layout (per core, contractions on the SBUF partition axis; ALL bulk
inputs pre-arranged on host to [128 partitions, contiguous-lines] so each
is ONE DMA with 128 fat descriptors instead of thousands of 1KB ones):
  xt  [128, 4*2048]   bf16 = q_x[b].T key-rolled, channel-chunked
  eb  [128, 2*16*512] bf16 = exp(bias[b,0]).T chunked [key128, ic, jc, q];
        softmax computed as exp(qk) * exp(bias), exact in fp32/bf16
  wt  [128, 4*1024]   bf16 = [wq.T/8 | wk.T | wv.T | wg.T] channel-chunked
  wot [128, 2*512]    bf16 = w_o[:, heads].T pair-major
  bg  [256, 1] f32 = 0.5*b_g[heads]  (gate via 0.5*tanh(0.5x+0.5bg)+0.5;
        Tanh shares the ACT "exp" table set -> no table swap)
  bsel [128,128] f32 = 0/1: row 64 -> out partitions 0:64, row 65 -> 64:128
        (engine writes at partition base 65 are illegal -> host constant)

Schedule. ACT exp of the scores ([128,1024] per step at ~1.0us, no dtype
speedup on ACT) is the hard floor: 64 steps ~= 67us. Everything else hides
under it or under the PE:
  - bf16 operands everywhere: halves DMA/SBUF/LDWEIGHTS traffic, DVE
    multiply runs in 2x mode (~0.69us), PE cycles unchanged
  - one flat software-pipelined stream of 64 (ic,hp,jc) steps: per step
    S-pair matmuls -> exp -> eb-multiply, with the O-accumulate matmuls
    emitted 2 steps behind so the in-order PE never waits on the exp
    pipeline; no pass-boundary stalls
  - PSUM = 2x 2-bank S slots + 2x 1-bank O accumulators + 2x 1-bank
    broadcast/out-proj slots = exactly 8 banks
  - projections: a minimal upfront set (K pair0, Q ic0 pair0, V j0/j1,
    gate) runs before the first step; the remaining ~21 groups ride a
    closure queue, interleaved 2-per-step into the early steps' PE slack
  - each pass's normalize + gate + out-projection is also carried as
    closures into the following steps; denominator row moves ride
    SBUF->SBUF DMAs (partition-base-65 writes are DMA-only anyway)
  - normalize is pair-wise: both heads' denominators assembled in one
    tile, one reciprocal, one fp32 bsel-matmul broadcast
All matmuls are plain 128x128 mode (mode switches drain the PE array).
"""

import sys

for _p in ("/opt/trn_rl_repo",):
    if _p not in sys.path:
        sys.path.insert(0, _p)

from collections import deque
from contextlib import ExitStack

import ml_dtypes
import numpy as np

import concourse.bass as bass  # noqa: F401
import concourse.mybir as mybir
import concourse.tile as tile
from concourse import bacc
from concourse.bass_utils import run_bass_kernel_spmd

# ---- problem dims (hardcoded per contest contract) ----
B, Q, CQ = 2, 2048, 512
H, D = 8, 64
P = 128
QL = 1024          # queries per core
EL = 256           # e-dims per core (4 heads x 64)
HL = 4             # heads per core
CC = CQ // P       # 4 contraction chunks over channels
EC = EL // P       # 2 head-pairs
NJ = Q // P        # 16 key chunks
NI = QL // 512     # 2 query chunks of 512

F32 = mybir.dt.float32
BF16 = mybir.dt.bfloat16
MUL = mybir.AluOpType.mult
ADD = mybir.AluOpType.add
EXP = mybir.ActivationFunctionType.Exp
TANH = mybir.ActivationFunctionType.Tanh

OFF_Q, OFF_K, OFF_V, OFF_G = 0, EL, 2 * EL, 3 * EL


def _emit(tc, xt, eb, wt, wot, bg, bsel, outp):
    nc = tc.nc

    with ExitStack() as ctx:
        const = ctx.enter_context(tc.tile_pool(name="const", bufs=1))
        esp = ctx.enter_context(tc.tile_pool(name="esp", bufs=4))
        ptp = ctx.enter_context(tc.tile_pool(name="ptp", bufs=6))
        workp = ctx.enter_context(tc.tile_pool(name="workp", bufs=2))
        psum = ctx.enter_context(tc.tile_pool(name="psum", bufs=2, space="PSUM"))

        # ---- ALL input DMAs first: nothing may delay the SP queue (the
        # GPSIMD memsets' first launch costs ~3us and used to push the
        # whole input stream to t=7.7us) ----
        bsel_sb = const.tile([P, P], F32, name="bsel_sb", tag="bsel_sb")
        nc.sync.dma_start(bsel_sb, bsel)
        bg_sb = const.tile([P, EC], F32, name="bg_sb", tag="bg_sb")
        nc.sync.dma_start(bg_sb, bg.rearrange("(o p) u -> p (o u)", p=P))
        woT_sb = const.tile([P, EC, CQ], BF16, name="woT_sb", tag="woT_sb")
        nc.sync.dma_start(woT_sb, wot.rearrange("p (o c) -> p o c", o=EC))
        wT_sb = const.tile([P, CC, 4 * EL], BF16, name="wT_sb", tag="wT_sb")
        nc.sync.dma_start(wT_sb, wt.rearrange("p (c n) -> p c n", c=CC))
        # xt j4-major: chunk j4 unblocks proj group j4 as soon as it lands
        xT_sb = const.tile([P, Q // 512, CC, 512], BF16, name="xT_sb", tag="xT_sb")
        xtr = xt.rearrange("p (j c q) -> p j c q", j=Q // 512, c=CC)
        for j4 in range(Q // 512):
            nc.sync.dma_start(xT_sb[:, j4, :, :], xtr[:, j4, :, :])
        # exp(bias), fully resident; SAME SP queue AFTER xt so the input
        # stream is strictly ordered (eb chunks land long before their step)
        ebt_sb = const.tile([P, NI, NJ, 512], BF16, name="ebt_sb", tag="ebt_sb")
        ebr = eb.rearrange("p (i j q) -> p i j q", i=NI, j=NJ)
        for ic in range(NI):
            for jq in range(4):
                nc.sync.dma_start(
                    ebt_sb[:, ic, jq * 4 : (jq + 1) * 4, :],
                    ebr[:, ic, jq * 4 : (jq + 1) * 4, :],
                )

        # ---- resident intermediates + on-chip constants (GPSIMD memsets
        # run concurrently with the input stream) ----
        rec_sbs = []
        for ri in range(2):
            rcb = const.tile([P, 512], F32, name=f"rec_sb{ri}", tag=f"rec_sb{ri}")
            nc.vector.memset(rcb, 0.0)
            rec_sbs.append(rcb)
        kT_sb = const.tile([P, EC, Q], BF16, name="kT_sb", tag="kT_sb")
        qTp_sb = const.tile([P, HL, QL], BF16, name="qTp_sb", tag="qTp_sb")
        nc.vector.memset(qTp_sb[64:128, 0::2, :], 0.0)
        nc.vector.memset(qTp_sb[0:64, 1::2, :], 0.0)
        gp_sb = const.tile([P, EC, QL], BF16, name="gp_sb", tag="gp_sb")
        og_sb = const.tile([P, EC, QL], BF16, name="og_sb", tag="og_sb")
        v_sb = const.tile([P, NJ, HL, D + 1], BF16, name="v_sb", tag="v_sb")
        nc.vector.memset(v_sb[:, :, :, D], 1.0)

        # ---- projection groups (each: 4 accum matmuls + drain) ----
        # rotate across all three psum tags: during the projection phase the
        # attention accumulator banks are idle, and a 6-slot rotation hides
        # the drain-semaphore latency that a 2-slot one exposes
        _ptag = {"n": 0}

        def _proj_ps():
            t = ("s", "op", "o")[_ptag["n"] % 3]
            _ptag["n"] += 1
            return psum.tile([P, 512], F32, tag=t, name="ps_proj")

        def proj_k(ec, j4):
            ps_k = _proj_ps()
            for c in range(CC):
                nc.tensor.matmul(
                    ps_k,
                    wT_sb[:, c, OFF_K + ec * P : OFF_K + (ec + 1) * P],
                    xT_sb[:, j4, c, :],
                    start=(c == 0),
                    stop=(c == CC - 1),
                )
            nc.vector.tensor_copy(
                out=kT_sb[:, ec, j4 * 512 : (j4 + 1) * 512], in_=ps_k
            )

        def proj_q(ic, ec):
            ps_q = _proj_ps()
            for c in range(CC):
                nc.tensor.matmul(
                    ps_q,
                    wT_sb[:, c, OFF_Q + ec * P : OFF_Q + (ec + 1) * P],
                    xT_sb[:, ic, c, :],
                    start=(c == 0),
                    stop=(c == CC - 1),
                )
            sl = slice(ic * 512, (ic + 1) * 512)
            nc.vector.tensor_copy(out=qTp_sb[0:64, 2 * ec, sl], in_=ps_q[0:64, :])
            nc.vector.tensor_copy(
                out=qTp_sb[64:128, 2 * ec + 1, sl], in_=ps_q[64:128, :]
            )

        def proj_v(jc):
            ps_v = _proj_ps()
            for c in range(CC):
                nc.tensor.matmul(
                    ps_v[:, :EL],
                    xT_sb[:, jc // 4, c, (jc % 4) * P : (jc % 4 + 1) * P],
                    wT_sb[:, c, OFF_V : OFF_V + EL],
                    start=(c == 0),
                    stop=(c == CC - 1),
                )
            nc.vector.tensor_copy(
                out=v_sb[:, jc, :, 0:D],
                in_=ps_v[:, :EL].rearrange("p (h d) -> p h d", h=HL),
            )

        def proj_g(ec, ic):
            ps_g = _proj_ps()
            for c in range(CC):
                nc.tensor.matmul(
                    ps_g,
                    wT_sb[:, c, OFF_G + ec * P : OFF_G + (ec + 1) * P],
                    xT_sb[:, ic, c, :],
                    start=(c == 0),
                    stop=(c == CC - 1),
                )
            nc.scalar.activation(
                gp_sb[:, ec, ic * 512 : (ic + 1) * 512],
                ps_g,
                TANH,
                bias=bg_sb[:, ec : ec + 1],
                scale=0.5,
            )

        # ---- phase 0: warmup burst (PE clock ramp) over the DMA head ----
        warm_ps = psum.tile([P, 2, 512], F32, tag="s", name="warm_ps")
        for wi in range(24):
            nc.tensor.matmul(
                warm_ps[:, 0, 0:P],
                bsel_sb,
                bsel_sb,
                start=(wi == 0),
                stop=(wi == 23),
            )
        warm_sb = workp.tile([P, P], F32, name="warm_sb", tag="warm")
        nc.vector.tensor_copy(out=warm_sb[:, 0:P], in_=warm_ps[:, 0, 0:P])

        # ---- phase 1: all projections upfront (PE-bound ~22us, fed by
        # the ordered chunk DMAs; ACT cannot absorb proj work during
        # attention anyway - the PE has only ~90ns/step slack there) ----
        proj_k(0, 0)
        proj_q(0, 0)
        for jc in range(4):
            proj_v(jc)
        proj_k(0, 1)
        for jc in range(4, 8):
            proj_v(jc)
        proj_k(0, 2)
        for jc in range(8, 12):
            proj_v(jc)
        proj_k(0, 3)
        for jc in range(12, 16):
            proj_v(jc)
        for j4 in range(Q // 512):
            proj_k(1, j4)
        proj_q(0, 1)
        proj_q(1, 0)
        proj_q(1, 1)
        proj_g(0, 0)
        proj_g(1, 0)
        proj_g(0, 1)
        proj_g(1, 1)
        nc.vector.tensor_scalar(gp_sb, gp_sb, 0.5, 0.5, MUL, ADD)

        # ---- phase 2: flat pipelined attention over 64 steps ----
        # partition-major output: per-partition lines are 8KB contiguous
        # (query-major rows made the out-DMA 512x 2KB descriptors, which
        # issue-rate-limited the endgame to ~8us); host un-permutes
        outr = outp.rearrange("p (o c) -> p o c", o=2 * HL)

        def make_tail(ic, hp, o_ps, parity, flush=False):
            """Normalize + gate closures for one finished (ic,hp) pass."""
            isl = slice(ic * 512, (ic + 1) * 512)
            rec_sb = rec_sbs[parity]
            o_pair = workp.tile([P, 512], F32, name="o_pair", tag="opair")
            wk = workp.tile([P, 512], F32, name="wk", tag="wk")
            recf = workp.tile([P, 512], F32, name="recf", tag="recf")
            ocp = workp.tile([P, 512], BF16, name="ocp", tag="ocp")
            bc_ps = psum.tile([P, 512], F32, tag="op", name="bc_ps")
            ops = []
            # zero wk rows 64:96 (legal base-64 band) before the denom
            # writes so the reciprocal reads deterministic data
            ops.append(lambda: nc.vector.memset(wk[64:96, :], 0.0))
            # even head -> o_pair rows 0:64 (denom parked in row 64).
            # In the flush ACT is idle: run the two drains concurrently.
            if flush:
                ops.append(
                    lambda: nc.scalar.copy(o_pair[0:65, :], o_ps[0])
                )
            else:
                ops.append(
                    lambda: nc.vector.tensor_copy(out=o_pair[0:65, :], in_=o_ps[0])
                )
            # odd head -> wk rows 0:64, its denom in wk[64]
            ops.append(lambda: nc.vector.tensor_copy(out=wk[0:65, :], in_=o_ps[1]))

            def _moves():
                # same SP queue: the row read of o_pair[64] (even denom ->
                # wk[96]) completes before the block write overwrites it
                nc.sync.dma_start(wk[96:97, :], o_pair[64:65, :])
                nc.sync.dma_start(o_pair[64:128, :], wk[0:64, :])

            ops.append(_moves)
            ops.append(
                lambda: nc.vector.reciprocal_approx_fast(
                    out=recf[0:97, :], in_=wk[0:97, :]
                )
            )
            # rec[64] = 1/denom_odd, rec[96] = 1/denom_even.
            # Mid-stream: DMA (latency hides under the step stream, DVE is
            # the scarce engine). Flush: DVE (no steps left to hide under).
            if flush:
                ops.append(
                    lambda: nc.vector.tensor_copy(
                        out=rec_sb[64:65, :], in_=recf[64:65, :]
                    )
                )
                ops.append(
                    lambda: nc.vector.tensor_copy(
                        out=rec_sb[96:97, :], in_=recf[96:97, :]
                    )
                )
            else:

                def _rows():
                    nc.sync.dma_start(rec_sb[64:65, :], recf[64:65, :])
                    nc.sync.dma_start(rec_sb[96:97, :], recf[96:97, :])

                ops.append(_rows)
            ops.append(
                lambda: nc.tensor.matmul(
                    bc_ps, bsel_sb, rec_sb, start=True, stop=True
                )
            )
            ops.append(lambda: nc.vector.tensor_tensor(ocp, bc_ps, o_pair, MUL))
            ops.append(
                lambda: nc.vector.tensor_tensor(
                    og_sb[:, hp, isl], ocp, gp_sb[:, hp, isl], MUL
                )
            )
            return ops

        def make_outproj(ic, flush=False):
            """Out-projection + store closures for one query block.
            All 4 chunks land in one tile, shipped by ONE fat-line DMA."""
            ops = []
            out_ic = workp.tile([P, 4, 512], F32, name="out_ic", tag="outic")
            for ip4 in range(4):
                ip = ic * 4 + ip4
                ps_o = psum.tile([P, 512], F32, tag="op", name="ps_o")

                def _mm(ps_o=ps_o, ip=ip):
                    for ec in range(EC):
                        nc.tensor.matmul(
                            ps_o,
                            og_sb[:, ec, ip * P : (ip + 1) * P],
                            woT_sb[:, ec, :],
                            start=(ec == 0),
                            stop=(ec == EC - 1),
                        )

                def _st(ps_o=ps_o, ip4=ip4):
                    if flush and ip4 % 2 == 0:
                        # ACT is idle in the flush; alternate with DVE so
                        # two drains run concurrently
                        nc.scalar.copy(out_ic[:, ip4, :], ps_o)
                    else:
                        nc.vector.tensor_copy(out=out_ic[:, ip4, :], in_=ps_o)

                ops.append(_mm)
                ops.append(_st)
                if flush and ip4 == 1:
                    # ship the first half early on the SP queue; the second
                    # half rides the (idle) ACT queue in parallel - a single
                    # 128-descriptor DMA issue-paces at ~90ns/descriptor
                    ops.append(
                        lambda: nc.sync.dma_start(
                            outr[:, ic * 4 : ic * 4 + 2, :], out_ic[:, 0:2, :]
                        )
                    )
            if flush:
                ops.append(
                    lambda: nc.scalar.dma_start(
                        outr[:, ic * 4 + 2 : ic * 4 + 4, :], out_ic[:, 2:4, :]
                    )
                )
            else:
                ops.append(
                    lambda: nc.sync.dma_start(
                        outr[:, ic * 4 : (ic + 1) * 4, :], out_ic
                    )
                )
            return ops

        tailq = deque()
        PASSES = [(0, 0), (0, 1), (1, 0), (1, 1)]
        steps = [(ic, hp, jc) for (ic, hp) in PASSES for jc in range(NJ)]
        DEPTH = 3
        o_ps_by_pass = {}
        pts = {}

        def emit_O(fs):
            ic, hp, jc = steps[fs]
            o_ps = o_ps_by_pass[(ic, hp)]
            pt = pts.pop(fs)
            for hh in range(2):
                nc.tensor.matmul(
                    o_ps[hh],
                    v_sb[:, jc, 2 * hp + hh, :],
                    pt[:, hh, :],
                    start=(jc == 0),
                    stop=(jc == NJ - 1),
                )
            if jc == NJ - 1:
                fl = ic == 1 and hp == 1
                tailq.extend(
                    make_tail(ic, hp, o_ps, parity=(2 * ic + hp) % 2, flush=fl)
                )
                if hp == 1:
                    tailq.extend(make_outproj(ic, flush=fl))

        for fs, (ic, hp, jc) in enumerate(steps):
            if jc == 0:
                o_ps_by_pass[(ic, hp)] = [
                    psum.tile(
                        [D + 1, 512], F32, tag="o", name=f"o_ps{hh}", bufs=2
                    )
                    for hh in range(2)
                ]
            isl = slice(ic * 512, (ic + 1) * 512)
            s_ps = psum.tile([P, 2, 512], F32, tag="s", name="s_ps")
            for hh in range(2):
                nc.tensor.matmul(
                    s_ps[:, hh, :],
                    kT_sb[:, hp, jc * P : (jc + 1) * P],
                    qTp_sb[:, 2 * hp + hh, isl],
                    start=True,
                    stop=True,
                )
            es = esp.tile([P, 2, 512], BF16, name="es", tag="es")
            nc.scalar.activation(es, s_ps, EXP)
            pt = ptp.tile([P, 2, 512], BF16, name="pt", tag="pt")
            ebb = ebt_sb[:, ic, jc, :][:, None, :].to_broadcast([P, 2, 512])
            nc.vector.tensor_tensor(pt, es, ebb, MUL)
            pts[fs] = pt
            # 1 closure/step keeps DVE under the ACT ceiling; the first
            # steps of a pass take 2 so the o_ps drains land before O(0)
            nops = 2 if jc < 6 else 1
            for _ in range(nops):
                if tailq:
                    tailq.popleft()()
            if fs >= DEPTH:
                emit_O(fs - DEPTH)
        for fs in range(len(steps) - DEPTH, len(steps)):
            emit_O(fs)
        # keep the PE clock from gating during the flush's dependency
        # latencies (HAM halves the clock after ~4us idle)
        warm2 = psum.tile([P, 2, 512], F32, tag="s", name="warm2")
        for wi in range(16):
            nc.tensor.matmul(
                warm2[:, 0, :],
                kT_sb[:, 0, 0:P],
                qTp_sb[:, 0, 0:512],
                start=(wi == 0),
                stop=(wi == 15),
            )
        while tailq:
            tailq.popleft()()


_CACHE = {}


def _get_nc():
    if "nc" not in _CACHE:
        nc = bacc.Bacc("TRN2", debug=False, enable_asserts=False)
        xt = nc.dram_tensor("xt_in", [P, CC * Q], BF16, kind="ExternalInput").ap()
        eb = nc.dram_tensor(
            "eb_in", [P, NI * NJ * 512], BF16, kind="ExternalInput"
        ).ap()
        wt = nc.dram_tensor("wt_in", [P, CC * 1024], BF16, kind="ExternalInput").ap()
        wot = nc.dram_tensor("wot_in", [P, EC * CQ], BF16, kind="ExternalInput").ap()
        bg = nc.dram_tensor("bg_in", [EL, 1], F32, kind="ExternalInput").ap()
        bsel = nc.dram_tensor("bsel_in", [P, P], F32, kind="ExternalInput").ap()
        outp = nc.dram_tensor("out", [P, 2 * HL * CQ], F32, kind="ExternalOutput").ap()
        with tile.TileContext(nc) as tc:
            _emit(tc, xt, eb, wt, wot, bg, bsel, outp)
        nc.compile()
        _CACHE["nc"] = nc
    return _CACHE["nc"]


LAST_RESULTS = None
BF = ml_dtypes.bfloat16
_BSEL = np.zeros((P, P), np.float32)
_BSEL[64, 64:128] = 1.0   # 1/denom_odd -> odd head rows
_BSEL[96, 0:64] = 1.0     # 1/denom_even -> even head rows


def _chunk128(a, nchunk):
    """[nchunk*128, L] -> [128, nchunk*L] partition-contiguous layout."""
    n, L = a.shape
    assert n == nchunk * P
    return a.reshape(nchunk, P, L).transpose(1, 0, 2).reshape(P, nchunk * L)


def kernel(q_x, kv_x, bias, w_qkv, w_o, b_o, w_g, b_g):
    global LAST_RESULTS
    q_x = np.asarray(q_x, np.float32)
    bias = np.asarray(bias, np.float32)
    w_qkv = np.asarray(w_qkv, np.float32)
    w_o = np.asarray(w_o, np.float32)
    b_o = np.asarray(b_o, np.float32)
    w_g = np.asarray(w_g, np.float32)
    b_g = np.asarray(b_g, np.float32)

    in_maps = []
    for core in range(8):
        b, qh, hq = core >> 2, (core >> 1) & 1, core & 1
        i0 = qh * QL
        esl = slice(hq * EL, (hq + 1) * EL)
        xTb = q_x[b].T  # [512, 2048]
        # roll keys so this core's queries are columns 0:QL
        xTp = np.concatenate([xTb[:, i0:], xTb[:, :i0]], axis=1)
        biasTb = bias[b, 0].T  # [keys, queries]
        ebp = np.exp(
            np.concatenate(
                [biasTb[i0:, i0 : i0 + QL], biasTb[:i0, i0 : i0 + QL]], axis=0
            )
        )
        # [2048 keys, 1024 q] -> [128, (ic, jc, 512)] partition-contiguous
        ebr = (
            ebp.reshape(NJ, P, NI, 512)
            .transpose(1, 2, 0, 3)
            .reshape(P, NI * NJ * 512)
        )
        wq = w_qkv[0:CQ][esl] * (1.0 / np.sqrt(D))
        wk = w_qkv[CQ : 2 * CQ][esl]
        wv = w_qkv[2 * CQ : 3 * CQ][esl]
        wg = w_g[esl]
        wTc = np.concatenate([wq.T, wk.T, wv.T, wg.T], axis=1)  # [512, 1024]
        woTc = w_o[:, esl].T  # [256, 512] pair-major rows
        bgc = (0.5 * b_g[esl]).reshape(EL, 1)
        in_maps.append(
            {
                "xt_in": np.ascontiguousarray(
                    xTp.reshape(CC, P, 4, 512)
                    .transpose(1, 2, 0, 3)
                    .reshape(P, CC * Q)
                ).astype(BF),
                "eb_in": np.ascontiguousarray(ebr).astype(BF),
                "wt_in": np.ascontiguousarray(_chunk128(wTc, CC)).astype(BF),
                "wot_in": np.ascontiguousarray(_chunk128(woTc, EC)).astype(BF),
                "bg_in": np.ascontiguousarray(bgc, np.float32),
                "bsel_in": _BSEL,
            }
        )

    nc = _get_nc()
    res = run_bass_kernel_spmd(nc, in_maps, core_ids=list(range(8)))
    LAST_RESULTS = res

    out = np.zeros((B, Q, CQ), np.float32)
    for core in range(8):
        b, qh = core >> 2, (core >> 1) & 1
        i0 = qh * QL
        oc = (
            res.results[core]["out"]
            .reshape(P, 2 * HL, CQ)
            .transpose(1, 0, 2)
            .reshape(QL, CQ)
        )
        out[b, i0 : i0 + QL] += oc
    out += b_o
    return out


# revision 22
# speedup vs baseline: 1.2598x; 1.0059x over previous
"""Trainium2 Bass kernel for nn_Attention_73289321939579.

Gated attention block (AlphaFold-style):
  qkv = q_x @ w_qkv.T ; q /= sqrt(64)
  scores = q k^T + bias ; attn = softmax(scores, keys)
  o = (attn @ v) * sigmoid(q_x @ w_g.T + b_g)
  out = o @ w_o.T + b_o

Sharding over 8 cores: core = b*4 + qh*2 + hq
  b  = batch (2)            -> data parallel
  qh = query half (2x1024)  -> bias/q sliced, output row-sliced
  hq = head quad (2x4 heads)-> tensor parallel; partial outputs summed on host

Device layout (per core, contractions on the SBUF partition axis; ALL bulk
inputs pre-arranged on host to [128 partitions, contiguous-lines] so each
is ONE DMA with 128 fat descriptors instead of thousands of 1KB ones):
  xt  [128, 4*2048]   bf16 = q_x[b].T key-rolled, channel-chunked
  eb  [128, 2*16*512] bf16 = exp(bias[b,0]).T chunked [key128, ic, jc, q];
        softmax computed as exp(qk) * exp(bias), exact in fp32/bf16
  wt  [128, 4*1024]   bf16 = [wq.T/8 | wk.T | wv.T | wg.T] channel-chunked
  wot [128, 2*512]    bf16 = w_o[:, heads].T pair-major
  bg  [256, 1] f32 = 0.5*b_g[heads]  (gate via 0.5*tanh(0.5x+0.5bg)+0.5;
        Tanh shares the ACT "exp" table set -> no table swap)
  bsel [128,128] f32 = 0/1: row 64 -> out partitions 0:64, row 65 -> 64:128
        (engine writes at partition base 65 are illegal -> host constant)

Schedule. ACT exp of the scores ([128,1024] per step at ~1.0us, no dtype
speedup on ACT) is the hard floor: 64 steps ~= 67us. Everything else hides
under it or under the PE:
  - bf16 operands everywhere: halves DMA/SBUF/LDWEIGHTS traffic, DVE
    multiply runs in 2x mode (~0.69us), PE cycles unchanged
  - one flat software-pipelined stream of 64 (ic,hp,jc) steps: per step
    S-pair matmuls -> exp -> eb-multiply, with the O-accumulate matmuls
    emitted 2 steps behind so the in-order PE never waits on the exp
    pipeline; no pass-boundary stalls
  - PSUM = 2x 2-bank S slots + 2x 1-bank O accumulators + 2x 1-bank
    broadcast/out-proj slots = exactly 8 banks
  - projections: a minimal upfront set (K pair0, Q ic0 pair0, V j0/j1,
    gate) runs before the first step; the remaining ~21 groups ride a
    closure queue, interleaved 2-per-step into the early steps' PE slack
  - each pass's normalize + gate + out-projection is also carried as
    closures into the following steps; denominator row moves ride
    SBUF->SBUF DMAs (partition-base-65 writes are DMA-only anyway)
  - normalize is pair-wise: both heads' denominators assembled in one
    tile, one reciprocal, one fp32 bsel-matmul broadcast
All matmuls are plain 128x128 mode (mode switches drain the PE array).
"""

import sys

for _p in ("/opt/trn_rl_repo",):
    if _p not in sys.path:
        sys.path.insert(0, _p)

from collections import deque
from contextlib import ExitStack

import ml_dtypes
import numpy as np

import concourse.bass as bass  # noqa: F401
import concourse.mybir as mybir
import concourse.tile as tile
from concourse import bacc
from concourse.bass_utils import run_bass_kernel_spmd

# ---- problem dims (hardcoded per contest contract) ----
B, Q, CQ = 2, 2048, 512
H, D = 8, 64
P = 128
QL = 1024          # queries per core
EL = 256           # e-dims per core (4 heads x 64)
HL = 4             # heads per core
CC = CQ // P       # 4 contraction chunks over channels
EC = EL // P       # 2 head-pairs
NJ = Q // P        # 16 key chunks
NI = QL // 512     # 2 query chunks of 512

F32 = mybir.dt.float32
BF16 = mybir.dt.bfloat16
MUL = mybir.AluOpType.mult
ADD = mybir.AluOpType.add
EXP = mybir.ActivationFunctionType.Exp
TANH = mybir.ActivationFunctionType.Tanh

OFF_Q, OFF_K, OFF_V, OFF_G = 0, EL, 2 * EL, 3 * EL


def _emit(tc, xt, eb, wt, wot, bg, bsel, outp):
    nc = tc.nc

    with ExitStack() as ctx:
        const = ctx.enter_context(tc.tile_pool(name="const", bufs=1))
        esp = ctx.enter_context(tc.tile_pool(name="esp", bufs=4))
        ptp = ctx.enter_context(tc.tile_pool(name="ptp", bufs=6))
        workp = ctx.enter_context(tc.tile_pool(name="workp", bufs=2))
        psum = ctx.enter_context(tc.tile_pool(name="psum", bufs=2, space="PSUM"))

        # ---- ALL input DMAs first: nothing may delay the SP queue (the
        # GPSIMD memsets' first launch costs ~3us and used to push the
        # whole input stream to t=7.7us) ----
        bsel_sb = const.tile([P, P], F32, name="bsel_sb", tag="bsel_sb")
        nc.sync.dma_start(bsel_sb, bsel)
        bg_sb = const.tile([P, EC], F32, name="bg_sb", tag="bg_sb")
        nc.sync.dma_start(bg_sb, bg.rearrange("(o p) u -> p (o u)", p=P))
        woT_sb = const.tile([P, EC, CQ], BF16, name="woT_sb", tag="woT_sb")
        nc.sync.dma_start(woT_sb, wot.rearrange("p (o c) -> p o c", o=EC))
        wT_sb = const.tile([P, CC, 4 * EL], BF16, name="wT_sb", tag="wT_sb")
        nc.sync.dma_start(wT_sb, wt.rearrange("p (c n) -> p c n", c=CC))
        # xt j4-major: chunk j4 unblocks proj group j4 as soon as it lands
        xT_sb = const.tile([P, Q // 512, CC, 512], BF16, name="xT_sb", tag="xT_sb")
        xtr = xt.rearrange("p (j c q) -> p j c q", j=Q // 512, c=CC)
        for j4 in range(Q // 512):
            nc.sync.dma_start(xT_sb[:, j4, :, :], xtr[:, j4, :, :])
        # exp(bias), fully resident; SAME SP queue AFTER xt so the input
        # stream is strictly ordered (eb chunks land long before their step)
        ebt_sb = const.tile([P, NI, NJ, 512], BF16, name="ebt_sb", tag="ebt_sb")
        ebr = eb.rearrange("p (i j q) -> p i j q", i=NI, j=NJ)
        for ic in range(NI):
            for jq in range(4):
                nc.sync.dma_start(
                    ebt_sb[:, ic, jq * 4 : (jq + 1) * 4, :],
                    ebr[:, ic, jq * 4 : (jq + 1) * 4, :],
                )

        # ---- resident intermediates + on-chip constants (GPSIMD memsets
        # run concurrently with the input stream) ----
        rec_sbs = []
        for ri in range(2):
            rcb = const.tile([P, 512], F32, name=f"rec_sb{ri}", tag=f"rec_sb{ri}")
            nc.vector.memset(rcb, 0.0)
            rec_sbs.append(rcb)
        kT_sb = const.tile([P, EC, Q], BF16, name="kT_sb", tag="kT_sb")
        qTp_sb = const.tile([P, HL, QL], BF16, name="qTp_sb", tag="qTp_sb")
        nc.vector.memset(qTp_sb[64:128, 0::2, :], 0.0)
        nc.vector.memset(qTp_sb[0:64, 1::2, :], 0.0)
        gp_sb = const.tile([P, EC, QL], BF16, name="gp_sb", tag="gp_sb")
        og_sb = const.tile([P, EC, QL], BF16, name="og_sb", tag="og_sb")
        v_sb = const.tile([P, NJ, HL, D + 1], BF16, name="v_sb", tag="v_sb")
        nc.vector.memset(v_sb[:, :, :, D], 1.0)

        # ---- projection groups (each: 4 accum matmuls + drain) ----
        # rotate across all three psum tags: during the projection phase the
        # attention accumulator banks are idle, and a 6-slot rotation hides
        # the drain-semaphore latency that a 2-slot one exposes
        _ptag = {"n": 0}

        def _proj_ps():
            t = ("s", "op", "o")[_ptag["n"] % 3]
            _ptag["n"] += 1
            return psum.tile([P, 512], F32, tag=t, name="ps_proj")

        def proj_k(ec, j4):
            ps_k = _proj_ps()
            for c in range(CC):
                nc.tensor.matmul(
                    ps_k,
                    wT_sb[:, c, OFF_K + ec * P : OFF_K + (ec + 1) * P],
                    xT_sb[:, j4, c, :],
                    start=(c == 0),
                    stop=(c == CC - 1),
                )
            nc.vector.tensor_copy(
                out=kT_sb[:, ec, j4 * 512 : (j4 + 1) * 512], in_=ps_k
            )

        def proj_q(ic, ec):
            ps_q = _proj_ps()
            for c in range(CC):
                nc.tensor.matmul(
                    ps_q,
                    wT_sb[:, c, OFF_Q + ec * P : OFF_Q + (ec + 1) * P],
                    xT_sb[:, ic, c, :],
                    start=(c == 0),
                    stop=(c == CC - 1),
                )
            sl = slice(ic * 512, (ic + 1) * 512)
            nc.vector.tensor_copy(out=qTp_sb[0:64, 2 * ec, sl], in_=ps_q[0:64, :])
            nc.vector.tensor_copy(
                out=qTp_sb[64:128, 2 * ec + 1, sl], in_=ps_q[64:128, :]
            )

        def proj_v(jc):
            ps_v = _proj_ps()
            for c in range(CC):
                nc.tensor.matmul(
                    ps_v[:, :EL],
                    xT_sb[:, jc // 4, c, (jc % 4) * P : (jc % 4 + 1) * P],
                    wT_sb[:, c, OFF_V : OFF_V + EL],
                    start=(c == 0),
                    stop=(c == CC - 1),
                )
            nc.vector.tensor_copy(
                out=v_sb[:, jc, :, 0:D],
                in_=ps_v[:, :EL].rearrange("p (h d) -> p h d", h=HL),
            )

        def proj_g(ec, ic):
            ps_g = _proj_ps()
            for c in range(CC):
                nc.tensor.matmul(
                    ps_g,
                    wT_sb[:, c, OFF_G + ec * P : OFF_G + (ec + 1) * P],
                    xT_sb[:, ic, c, :],
                    start=(c == 0),
                    stop=(c == CC - 1),
                )
            nc.scalar.activation(
                gp_sb[:, ec, ic * 512 : (ic + 1) * 512],
                ps_g,
                TANH,
                bias=bg_sb[:, ec : ec + 1],
                scale=0.5,
            )

        # ---- phase 0: warmup burst (PE clock ramp) over the DMA head ----
        warm_ps = psum.tile([P, 2, 512], F32, tag="s", name="warm_ps")
        for wi in range(24):
            nc.tensor.matmul(
                warm_ps[:, 0, 0:P],
                bsel_sb,
                bsel_sb,
                start=(wi == 0),
                stop=(wi == 23),
            )
        warm_sb = workp.tile([P, P], F32, name="warm_sb", tag="warm")
        nc.vector.tensor_copy(out=warm_sb[:, 0:P], in_=warm_ps[:, 0, 0:P])

        # ---- phase 1: all projections upfront (PE-bound ~22us, fed by
        # the ordered chunk DMAs; ACT cannot absorb proj work during
        # attention anyway - the PE has only ~90ns/step slack there) ----
        proj_k(0, 0)
        proj_q(0, 0)
        for jc in range(4):
            proj_v(jc)
        proj_k(0, 1)
        for jc in range(4, 8):
            proj_v(jc)
        proj_k(0, 2)
        for jc in range(8, 12):
            proj_v(jc)
        proj_k(0, 3)
        for jc in range(12, 16):
            proj_v(jc)
        for j4 in range(Q // 512):
            proj_k(1, j4)
        proj_q(0, 1)
        proj_q(1, 0)
        proj_q(1, 1)
        proj_g(0, 0)
        proj_g(1, 0)
        proj_g(0, 1)
        proj_g(1, 1)
        nc.vector.tensor_scalar(gp_sb, gp_sb, 0.5, 0.5, MUL, ADD)

        # ---- phase 2: flat pipelined attention over 64 steps ----
        # partition-major output: per-partition lines are 8KB contiguous
        # (query-major rows made the out-DMA 512x 2KB descriptors, which
        # issue-rate-limited the endgame to ~8us); host un-permutes
        outr = outp.rearrange("p (o c) -> p o c", o=2 * HL)

        def make_tail(ic, hp, o_ps, parity, flush=False):
            """Normalize + gate closures for one finished (ic,hp) pass."""
            isl = slice(ic * 512, (ic + 1) * 512)
            rec_sb = rec_sbs[parity]
            o_pair = workp.tile([P, 512], F32, name="o_pair", tag="opair")
            wk = workp.tile([P, 512], F32, name="wk", tag="wk")
            recf = workp.tile([P, 512], F32, name="recf", tag="recf")
            ocp = workp.tile([P, 512], BF16, name="ocp", tag="ocp")
            bc_ps = psum.tile([P, 512], F32, tag="op", name="bc_ps")
            ops = []
            # zero wk rows 64:96 (legal base-64 band) before the denom
            # writes so the reciprocal reads deterministic data
            ops.append(lambda: nc.vector.memset(wk[64:96, :], 0.0))
            # even head -> o_pair rows 0:64 (denom parked in row 64).
            # In the flush ACT is idle: run the two drains concurrently.
            if flush:
                ops.append(
                    lambda: nc.scalar.copy(o_pair[0:65, :], o_ps[0])
                )
            else:
                ops.append(
                    lambda: nc.vector.tensor_copy(out=o_pair[0:65, :], in_=o_ps[0])
                )
            # odd head -> wk rows 0:64, its denom in wk[64]
            ops.append(lambda: nc.vector.tensor_copy(out=wk[0:65, :], in_=o_ps[1]))

            def _moves():
                # same SP queue: the row read of o_pair[64] (even denom ->
                # wk[96]) completes before the block write overwrites it
                nc.sync.dma_start(wk[96:97, :], o_pair[64:65, :])
                nc.sync.dma_start(o_pair[64:128, :], wk[0:64, :])

            ops.append(_moves)
            ops.append(
                lambda: nc.vector.reciprocal_approx_fast(
                    out=recf[0:97, :], in_=wk[0:97, :]
                )
            )
            # rec[64] = 1/denom_odd, rec[96] = 1/denom_even.
            # Mid-stream: DMA (latency hides under the step stream, DVE is
            # the scarce engine). Flush: DVE (no steps left to hide under).
            if flush:
                ops.append(
                    lambda: nc.vector.tensor_copy(
                        out=rec_sb[64:65, :], in_=recf[64:65, :]
                    )
                )
                ops.append(
                    lambda: nc.vector.tensor_copy(
                        out=rec_sb[96:97, :], in_=recf[96:97, :]
                    )
                )
            else:

                def _rows():
                    nc.sync.dma_start(rec_sb[64:65, :], recf[64:65, :])
                    nc.sync.dma_start(rec_sb[96:97, :], recf[96:97, :])

                ops.append(_rows)
            ops.append(
                lambda: nc.tensor.matmul(
                    bc_ps, bsel_sb, rec_sb, start=True, stop=True
                )
            )
            ops.append(lambda: nc.vector.tensor_tensor(ocp, bc_ps, o_pair, MUL))
            ops.append(
                lambda: nc.vector.tensor_tensor(
                    og_sb[:, hp, isl], ocp, gp_sb[:, hp, isl], MUL
                )
            )
            return ops

        def make_outproj(ic, flush=False):
            """Out-projection + store closures for one query block.
            All 4 chunks land in one tile, shipped by ONE fat-line DMA."""
            ops = []
            out_ic = workp.tile([P, 4, 512], F32, name="out_ic", tag="outic")
            for ip4 in range(4):
                ip = ic * 4 + ip4
                ps_o = psum.tile([P, 512], F32, tag="op", name="ps_o")

                def _mm(ps_o=ps_o, ip=ip):
                    for ec in range(EC):
                        nc.tensor.matmul(
                            ps_o,
                            og_sb[:, ec, ip * P : (ip + 1) * P],
                            woT_sb[:, ec, :],
                            start=(ec == 0),
                            stop=(ec == EC - 1),
                        )

                def _st(ps_o=ps_o, ip4=ip4):
                    if flush and ip4 % 2 == 0:
                        # ACT is idle in the flush; alternate with DVE so
                        # two drains run concurrently
                        nc.scalar.copy(out_ic[:, ip4, :], ps_o)
                    else:
                        nc.vector.tensor_copy(out=out_ic[:, ip4, :], in_=ps_o)

                ops.append(_mm)
                ops.append(_st)
                if flush and ip4 == 1:
                    # ship the first half early on the SP queue; the second
                    # half rides the (idle) ACT queue in parallel - a single
                    # 128-descriptor DMA issue-paces at ~90ns/descriptor
                    ops.append(
                        lambda: nc.sync.dma_start(
                            outr[:, ic * 4 : ic * 4 + 2, :], out_ic[:, 0:2, :]
                        )
                    )
            if flush:
                ops.append(
                    lambda: nc.scalar.dma_start(
                        outr[:, ic * 4 + 2 : ic * 4 + 4, :], out_ic[:, 2:4, :]
                    )
                )
            else:
                ops.append(
                    lambda: nc.sync.dma_start(
                        outr[:, ic * 4 : (ic + 1) * 4, :], out_ic
                    )
                )
            return ops

        tailq = deque()
        PASSES = [(0, 0), (0, 1), (1, 0), (1, 1)]
        steps = [(ic, hp, jc) for (ic, hp) in PASSES for jc in range(NJ)]
        DEPTH = 4
        o_ps_by_pass = {}
        pts = {}

        def emit_O(fs):
            ic, hp, jc = steps[fs]
            o_ps = o_ps_by_pass[(ic, hp)]
            pt = pts.pop(fs)
            for hh in range(2):
                nc.tensor.matmul(
                    o_ps[hh],
                    v_sb[:, jc, 2 * hp + hh, :],
                    pt[:, hh, :],
                    start=(jc == 0),
                    stop=(jc == NJ - 1),
                )
            if jc == NJ - 1:
                fl = ic == 1 and hp == 1
                tailq.extend(
                    make_tail(ic, hp, o_ps, parity=(2 * ic + hp) % 2, flush=fl)
                )
                if hp == 1:
                    tailq.extend(make_outproj(ic, flush=fl))

        for fs, (ic, hp, jc) in enumerate(steps):
            if jc == 0:
                o_ps_by_pass[(ic, hp)] = [
                    psum.tile(
                        [D + 1, 512], F32, tag="o", name=f"o_ps{hh}", bufs=2
                    )
                    for hh in range(2)
                ]
            isl = slice(ic * 512, (ic + 1) * 512)
            s_ps = psum.tile([P, 2, 512], F32, tag="s", name="s_ps")
            for hh in range(2):
                nc.tensor.matmul(
                    s_ps[:, hh, :],
                    kT_sb[:, hp, jc * P : (jc + 1) * P],
                    qTp_sb[:, 2 * hp + hh, isl],
                    start=True,
                    stop=True,
                )
            es = esp.tile([P, 2, 512], BF16, name="es", tag="es")
            nc.scalar.activation(es, s_ps, EXP)
            pt = ptp.tile([P, 2, 512], BF16, name="pt", tag="pt")
            ebb = ebt_sb[:, ic, jc, :][:, None, :].to_broadcast([P, 2, 512])
            nc.vector.tensor_tensor(pt, es, ebb, MUL)
            pts[fs] = pt
            # 1 closure/step keeps DVE under the ACT ceiling; the first
            # steps of a pass take 2 so the o_ps drains land before O(0)
            # (3x2 + 13x1 = 19 exactly covers a tail+outproj queue without
            # clustering DVE closure work against the eb-multiply)
            nops = 2 if jc < 3 else 1
            for _ in range(nops):
                if tailq:
                    tailq.popleft()()
            if fs >= DEPTH:
                emit_O(fs - DEPTH)
        for fs in range(len(steps) - DEPTH, len(steps)):
            emit_O(fs)
        # keep the PE clock from gating during the flush's dependency
        # latencies (HAM halves the clock after ~4us idle)
        warm2 = psum.tile([P, 2, 512], F32, tag="s", name="warm2")
        for wi in range(16):
            nc.tensor.matmul(
                warm2[:, 0, :],
                kT_sb[:, 0, 0:P],
                qTp_sb[:, 0, 0:512],
                start=(wi == 0),
                stop=(wi == 15),
            )
        while tailq:
            tailq.popleft()()


_CACHE = {}


def _get_nc():
    if "nc" not in _CACHE:
        nc = bacc.Bacc("TRN2", debug=False, enable_asserts=False)
        xt = nc.dram_tensor("xt_in", [P, CC * Q], BF16, kind="ExternalInput").ap()
        eb = nc.dram_tensor(
            "eb_in", [P, NI * NJ * 512], BF16, kind="ExternalInput"
        ).ap()
        wt = nc.dram_tensor("wt_in", [P, CC * 1024], BF16, kind="ExternalInput").ap()
        wot = nc.dram_tensor("wot_in", [P, EC * CQ], BF16, kind="ExternalInput").ap()
        bg = nc.dram_tensor("bg_in", [EL, 1], F32, kind="ExternalInput").ap()
        bsel = nc.dram_tensor("bsel_in", [P, P], F32, kind="ExternalInput").ap()
        outp = nc.dram_tensor("out", [P, 2 * HL * CQ], F32, kind="ExternalOutput").ap()
        with tile.TileContext(nc) as tc:
            _emit(tc, xt, eb, wt, wot, bg, bsel, outp)
        nc.compile()
        _CACHE["nc"] = nc
    return _CACHE["nc"]


LAST_RESULTS = None
BF = ml_dtypes.bfloat16
_BSEL = np.zeros((P, P), np.float32)
_BSEL[64, 64:128] = 1.0   # 1/denom_odd -> odd head rows
_BSEL[96, 0:64] = 1.0     # 1/denom_even -> even head rows


def _chunk128(a, nchunk):
    """[nchunk*128, L] -> [128, nchunk*L] partition-contiguous layout."""
    n, L = a.shape
    assert n == nchunk * P
    return a.reshape(nchunk, P, L).transpose(1, 0, 2).reshape(P, nchunk * L)


def kernel(q_x, kv_x, bias, w_qkv, w_o, b_o, w_g, b_g):
    global LAST_RESULTS
    q_x = np.asarray(q_x, np.float32)
    bias = np.asarray(bias, np.float32)
    w_qkv = np.asarray(w_qkv, np.float32)
    w_o = np.asarray(w_o, np.float32)
    b_o = np.asarray(b_o, np.float32)
    w_g = np.asarray(w_g, np.float32)
    b_g = np.asarray(b_g, np.float32)

    in_maps = []
    for core in range(8):
        b, qh, hq = core >> 2, (core >> 1) & 1, core & 1
        i0 = qh * QL
        esl = slice(hq * EL, (hq + 1) * EL)
        xTb = q_x[b].T  # [512, 2048]
        # roll keys so this core's queries are columns 0:QL
        xTp = np.concatenate([xTb[:, i0:], xTb[:, :i0]], axis=1)
        biasTb = bias[b, 0].T  # [keys, queries]
        ebp = np.exp(
            np.concatenate(
                [biasTb[i0:, i0 : i0 + QL], biasTb[:i0, i0 : i0 + QL]], axis=0
            )
        )
        # [2048 keys, 1024 q] -> [128, (ic, jc, 512)] partition-contiguous
        ebr = (
            ebp.reshape(NJ, P, NI, 512)
            .transpose(1, 2, 0, 3)
            .reshape(P, NI * NJ * 512)
        )
        wq = w_qkv[0:CQ][esl] * (1.0 / np.sqrt(D))
        wk = w_qkv[CQ : 2 * CQ][esl]
        wv = w_qkv[2 * CQ : 3 * CQ][esl]
        wg = w_g[esl]
        wTc = np.concatenate([wq.T, wk.T, wv.T, wg.T], axis=1)  # [512, 1024]
        woTc = w_o[:, esl].T  # [256, 512] pair-major rows
        bgc = (0.5 * b_g[esl]).reshape(EL, 1)
        in_maps.append(
            {
                "xt_in": np.ascontiguousarray(
                    xTp.reshape(CC, P, 4, 512)
                    .transpose(1, 2, 0, 3)
                    .reshape(P, CC * Q)
                ).astype(BF),
                "eb_in": np.ascontiguousarray(ebr).astype(BF),
                "wt_in": np.ascontiguousarray(_chunk128(wTc, CC)).astype(BF),
                "wot_in": np.ascontiguousarray(_chunk128(woTc, EC)).astype(BF),
                "bg_in": np.ascontiguousarray(bgc, np.float32),
                "bsel_in": _BSEL,
            }
        )

    nc = _get_nc()
    res = run_bass_kernel_spmd(nc, in_maps, core_ids=list(range(8)))
    LAST_RESULTS = res

    out = np.zeros((B, Q, CQ), np.float32)
    for core in range(8):
        b, qh = core >> 2, (core >> 1) & 1
        i0 = qh * QL
        oc = (
            res.results[core]["out"]
            .reshape(P, 2 * HL, CQ)
            .transpose(1, 0, 2)
            .reshape(QL, CQ)
        )
        out[b, i0 : i0 + QL] += oc
    out += b_o
    return out
